# revision 32
# baseline (speedup 1.0000x reference)
"""Trainium2 Bass kernel for single-head causal attention.

Problem: x[4,2048,1024] f32; Wq/Wk/Wv [1024,1024] (torch Linear layout, y = x@W.T).
  q,k,v = x@W.T ; scores = q@k.T (causal masked, scaled 1/sqrt(1024)) ;
  out = softmax(scores)@v.

Weight folding: scores = xq (Wq^T Wk) xk^T, so with M := 32*(Wq^T Wk)
precomputed on the host the K projection disappears -- x^T itself is the key
matrix. Likewise out = w @ x @ Wv^T, so the V projection collapses to a
per-slot (w.x) @ Wv^T postmultiply.

fp8 everywhere the error averages out: the softmax temperature (1/32) makes
the score path error-tolerant, and PV pass A's quantization noise is iid
across keys so it averages down by sqrt(n_keys). Both run fp8-e4m3 with
DoubleRow perf mode (2 k-tiles per pass, ~1.5x bf16). Pass B (contraction
over d with fresh noise per d: no averaging) stays bf16.

Early rows have few keys -> no averaging, so slot 0 (each core's first query
block = global rows 0..255, 256-key causal extent) runs its whole score +
pass-A path in bf16: a small bf16 q-projection for its 128 queries, bf16
keys, bf16 pass A. This drops max-rel-err from ~1.4e-2 to ~4e-3 (gate 2e-2).

Softmax drops max-subtraction: scores'/1024 = scores/32 is bounded ~1.8 so
exp can't overflow; masked entries underflow to 0; exp runs per-chunk
straight from PSUM with accumulated partial sums.

Sharding: 2 cores per batch, zig-zag query blocks (identical causal extents
[1,8,2,7,3,6,4,5] chunks of 256 on every core -> one SPMD program).

Per-core pipeline (fp32 PSUM accumulation):
  1. qMT = (xq @ M)^T in fp8 DoubleRow; qMT stored fp8.
  2. QK (slots longest-first) fp8 DoubleRow; mask in-place on PSUM edge
     chunk; exp from PSUM on ACT with accumulated sums; weights stored fp8
     (slot 0: bf16 mini-qproj + bf16 QK, weights bf16).
  3. PV pass A fp8 DoubleRow: fp8 PE-transposes of weight block pairs +
     (w @ x8) accumulation, prev slot's (wx) transposes interleaved.
  4. PV pass B bf16: (wx)^T @ Wv^T, 1/sum fused into PSUM->SBUF, DMA out.
"""

from contextlib import ExitStack

import ml_dtypes
import numpy as np

import concourse.mybir as mybir
import concourse.tile as tile
from concourse import bacc
from concourse.bass_utils import run_bass_kernel_spmd

B, S, D, E = 4, 2048, 1024, 1024
P = 128
N_CORES = 8
DT = D // P          # 8 d-tiles (contraction)
DP = DT // 2         # 4 d-tile PAIRS (fp8 DoubleRow contracts 2 tiles/pass)
SQ = S // 2          # 1024 query rows per core
KCH = 256            # causal-length granularity (key chunk)
NSLOT = SQ // P      # 8 query slots per core

QCH = [512, 512]     # xqT chunking (DoubleRow wants N>=512 passes)
assert sum(QCH) == SQ

# zig-zag query-block assignment: both cores' slots have identical causal
# chunk counts CJ, so one SPMD program serves all cores.
QBLOCKS = [[0, 15, 2, 13, 4, 11, 6, 9], [1, 14, 3, 12, 5, 10, 7, 8]]
CJ = [(b + 1 + 1) // 2 for b in QBLOCKS[0]]  # [1,8,2,7,3,6,4,5]
assert CJ == [(b + 1 + 1) // 2 for b in QBLOCKS[1]]
SLOT_ORDER = sorted(range(NSLOT), key=lambda j: -CJ[j])  # longest first

F32 = mybir.dt.float32
BF16 = mybir.dt.bfloat16
FP8 = mybir.dt.float8e4
DR = mybir.MatmulPerfMode.DoubleRow
AX = mybir.AxisListType.X
EXP = mybir.ActivationFunctionType.Exp
EXP_SCALE = 1.0 / 1024.0   # (1/32 softmax temp) * (1/32 host M-scale)
MASK_VAL = -1.0e9
WPIECES = [(0, 1), (1, 2), (2, 4), (4, 8)]  # M DMA split over out-tiles


def build_kernel():
    nc = bacc.Bacc(
        "TRN2",
        target_bir_lowering=False,
        debug=False,
        num_devices=N_CORES,
        dynamic_dma_scratch_size=64,
    )
    xT_d = nc.dram_tensor("xT", [P, DT, S], FP8, kind="ExternalInput")
    xn_d = nc.dram_tensor("xn", [P, S // P, D], FP8, kind="ExternalInput")
    xqT_d = nc.dram_tensor("xqT", [P, DT, SQ], FP8, kind="ExternalInput")
    m_d = nc.dram_tensor("MT", [P, DT, DT, P], FP8, kind="ExternalInput")
    wv_d = nc.dram_tensor("WvT", [P, DT, E], BF16, kind="ExternalInput")
    msk_d = nc.dram_tensor("masks", [P, NSLOT, KCH], BF16, kind="ExternalInput")
    # bf16 sidecar for slot 0 (rows 0..255): M, first 128 gathered queries,
    # first 256 keys (transposed + natural)
    m16_d = nc.dram_tensor("MT16", [P, DT, DT, P], BF16, kind="ExternalInput")
    xq16_d = nc.dram_tensor("xq16", [P, DT, P], BF16, kind="ExternalInput")
    xT16_d = nc.dram_tensor("xT16", [P, DT, KCH], BF16, kind="ExternalInput")
    xn16_d = nc.dram_tensor("xn16", [P, 2, D], BF16, kind="ExternalInput")
    out_d = nc.dram_tensor("out", [SQ, E], F32, kind="ExternalOutput")

    with tile.TileContext(nc) as tc, ExitStack() as ctx:
        # persistent tensors (right side). Bulk inputs are split into tiles
        # of <=4KB/partition so every dma_start is a single segment with its
        # own semaphore -- larger DMAs get chopped into semaphore-chained
        # segments that occupy the issuing queue until the transfer lands,
        # head-of-line blocking everything behind them (the xbar transposes).
        kqv = ctx.enter_context(tc.tile_pool(name="kqv", bufs=1, side="right"))
        # keys x^T by DoubleRow d-pair: xTt[dp][p, s, k] = x[k, (2dp+s)*128+p]
        xTt = [kqv.tile([P, 2, S], FP8, tag=f"xT{dp}", name=f"xT{dp}") for dp in range(DP)]
        # x natural by key-block group: xnt[a][p, b, d] = x[(4a+b)*128+p, d]
        xnt = [kqv.tile([P, 4, D], FP8, tag=f"xn{a}", name=f"xn{a}") for a in range(4)]
        qMT = kqv.tile([P, DT, SQ], FP8, tag="qMT")      # (xq M)^T (fp8)
        # WvT by d-pair: wvt[h][p, s, e] = Wv[e, (2h+s)*128+p]
        wvt = [kqv.tile([P, 2, E], BF16, tag=f"wv{h}", name=f"wv{h}") for h in range(DP)]
        msk = kqv.tile([P, NSLOT, KCH], BF16, tag="msk")
        # M16 by jt-pair: m16t[g][p, s, d, el] = M[d*128+p, (2g+s)*128+el]
        m16t = [kqv.tile([P, 2, DT, P], BF16, tag=f"m16{g}", name=f"m16{g}") for g in range(DP)]
        xq16 = kqv.tile([P, DT, P], BF16, tag="xq16")
        xT16 = kqv.tile([P, DT, KCH], BF16, tag="xT16")
        xn16 = kqv.tile([P, 2, D], BF16, tag="xn16")
        qMT16 = kqv.tile([P, DT, P], BF16, tag="qMT16")

        # ---------------- folded q projection ----------------
        with (
            tc.tile_pool(name="wpool", bufs=1) as wpool,
            tc.tile_pool(name="xpool", bufs=2) as xpool,
            tc.tile_pool(name="pps", bufs=6, space="PSUM") as pps,
        ):
            # HAM warm-up: dummy matmuls on a zeroed tile fill the DMA-init
            # dead zone and un-throttle the PE clock before real work
            warm = xpool.tile([P, 512], BF16, tag="warm", name="warm", bufs=1)
            nc.gpsimd.memset(warm[:], 0.0)
            wps = pps.tile([P, 512], F32, tag="wps", name="wps", bufs=1)
            for _ in range(10):
                nc.tensor.matmul(
                    wps[:], lhsT=warm[:, 0:P], rhs=warm[:], start=True, stop=True
                )
            for _ in range(6):
                nc.tensor.matmul(
                    wps[:, 0:256],
                    lhsT=warm[:, 0:P],
                    rhs=warm[:, 0:256],
                    start=True,
                    stop=True,
                )

            m_sb = wpool.tile([P, DT, DT, P], FP8, tag="M", name="m_sb")
            lo, hi = WPIECES[0]
            nc.sync.dma_start(m_sb[:, lo:hi], m_d[:, lo:hi])
            xqc = []
            t0 = 0
            for ci, csz in enumerate(QCH):
                xc = xpool.tile([P, DT, 512], FP8, tag="x", name="xc")
                nc.sync.dma_start(xc[:, :, 0:csz], xqT_d[:, :, t0 : t0 + csz])
                xqc.append(xc)
                t0 += csz
                if ci == 0:
                    for lo, hi in WPIECES[1:]:
                        nc.sync.dma_start(m_sb[:, lo:hi], m_d[:, lo:hi])
            # bulk streaming inputs, ordered by first use in the attention
            # phases: xT (QK), masks, slot-0 bf16 sidecar (mini-qproj sits
            # mid-QK), xn (pass A), WvT (pass B) -- all single-segment DMAs
            for dp in range(DP):
                nc.sync.dma_start(xTt[dp][:], xT_d[:, 2 * dp : 2 * dp + 2])
            nc.sync.dma_start(msk[:], msk_d[:])
            for g in range(DP):
                nc.sync.dma_start(m16t[g][:], m16_d[:, 2 * g : 2 * g + 2])
            # the two small sidecar loads ride the ACT queue: warms the ACT
            # DGE early so the weight-transpose xbar DMAs (also ACT-issued,
            # from ~28us) don't pay its init, and trims the sync queue
            nc.scalar.dma_start(xq16[:], xq16_d[:])
            nc.scalar.dma_start(xT16[:], xT16_d[:])
            for a in range(4):
                nc.sync.dma_start(xnt[a][:], xn_d[:, 4 * a : 4 * a + 4])
            nc.sync.dma_start(xn16[:], xn16_d[:])
            for h in range(DP):
                nc.sync.dma_start(wvt[h][:], wv_d[:, 2 * h : 2 * h + 2])

            t0 = 0
            for ci, csz in enumerate(QCH):
                xc = xqc[ci]
                for j_t in range(DT):
                    ps = pps.tile([P, 512], F32, tag="ps", name="ps")
                    for dp in range(DP):
                        nc.tensor.matmul(
                            ps[:, 0:csz],
                            lhsT=m_sb[:, j_t, 2 * dp : 2 * dp + 2, :],
                            rhs=xc[:, 2 * dp : 2 * dp + 2, 0:csz],
                            perf_mode=DR,
                            start=(dp == 0),
                            stop=(dp == DP - 1),
                        )
                    nc.scalar.copy(qMT[:, j_t, t0 : t0 + csz], ps[:, 0:csz])
                t0 += csz

        # ---------------- attention ----------------
        with (
            tc.tile_pool(name="apool", bufs=2) as apool,
            tc.tile_pool(name="wtpool", bufs=4) as wtpool,
            tc.tile_pool(name="wxtpool", bufs=NSLOT) as wxtpool,
            tc.tile_pool(name="stpool", bufs=NSLOT, side="right") as stpool,
        ):
            def emit_scores(j):
                """QK + mask + exp + sum for slot j; transposes the weight
                tile through the DMA xbar (and quantizes to fp8 on DVE for
                the DoubleRow pass-A lhsT). Slot 0 -> bf16 path."""
                C = CJ[j]
                L = C * KCH
                nkb = C * KCH // P
                st = stpool.tile([P, 8], F32, tag="st", name="st")
                if C == 1:
                    # bf16 mini-qproj for slot-0's 128 queries (queued at the
                    # QK tail: SLOT_ORDER puts this slot last)
                    for j_t in range(DT):
                        ps = qkps.tile([P, 512], F32, tag="qk", name="mq")
                        for d in range(DT):
                            nc.tensor.matmul(
                                ps[:, 0:P],
                                lhsT=m16t[j_t // 2][:, j_t % 2, d, :],
                                rhs=xq16[:, d, :],
                                start=(d == 0),
                                stop=(d == DT - 1),
                            )
                        nc.scalar.copy(qMT16[:, j_t, :], ps[:, 0:P])
                    wts = apool.tile(
                        [P, KCH], BF16, tag="wts16", name="wts16", bufs=1
                    )
                    ps = qkps.tile([P, 512], F32, tag="qk", name="qk")
                    for d in range(DT):
                        nc.tensor.matmul(
                            ps[:, 0:KCH],
                            lhsT=qMT16[:, d, :],
                            rhs=xT16[:, d, :],
                            start=(d == 0),
                            stop=(d == DT - 1),
                        )
                    nc.vector.tensor_add(
                        ps[:, 0:KCH], ps[:, 0:KCH], msk[:, j, :]
                    )
                    nc.scalar.activation(
                        wts[:, 0:KCH],
                        ps[:, 0:KCH],
                        EXP,
                        scale=EXP_SCALE,
                        accum_out=st[:, 0:1],
                    )
                    nc.vector.reciprocal(st[:, 7:8], st[:, 0:1])
                    wT = wtpool.tile(
                        [P, 2, P], BF16, tag="wt16s", name="wt16s", bufs=1
                    )
                    nc.scalar.dma_start(wT[:], wts[:, 0:KCH], transpose=True)
                    return st, wT

                groups = [(g * 512, 512) for g in range(C // 2)]
                if C % 2:
                    groups.append(((C // 2) * 512, 256))
                wts = apool.tile(
                    [P, S], BF16, tag="wts8", name="wts8", bufs=NSLOT - 1
                )
                nch = len(groups)
                for ci, (k0, ksz) in enumerate(groups):
                    ps = qkps.tile([P, 512], F32, tag="qk", name="qk")
                    for dp in range(DP):
                        nc.tensor.matmul(
                            ps[:, 0:ksz],
                            lhsT=qMT[:, 2 * dp : 2 * dp + 2, j * P : (j + 1) * P],
                            rhs=xTt[dp][:, :, k0 : k0 + ksz],
                            perf_mode=DR,
                            start=(dp == 0),
                            stop=(dp == DP - 1),
                        )
                    if k0 + ksz == L:
                        # causal edge: host mask covers the last 256 keys
                        nc.vector.tensor_add(
                            ps[:, ksz - 256 : ksz],
                            ps[:, ksz - 256 : ksz],
                            msk[:, j, :],
                        )
                    nc.scalar.activation(
                        wts[:, k0 : k0 + ksz],
                        ps[:, 0:ksz],
                        EXP,
                        scale=EXP_SCALE,
                        accum_out=st[:, ci : ci + 1],
                    )
                if nch > 1:
                    nc.vector.tensor_reduce(
                        st[:, 6:7], st[:, 0:nch], axis=AX, op=mybir.AluOpType.add
                    )
                    nc.vector.reciprocal(st[:, 7:8], st[:, 6:7])
                else:
                    nc.vector.reciprocal(st[:, 7:8], st[:, 0:1])
                wTb = wtpool.tile(
                    [P, nkb, P], BF16, tag=f"wtb{j}", name="wtb", bufs=1
                )
                nc.scalar.dma_start(wTb[:], wts[:, 0:L], transpose=True)
                wT = wtpool.tile(
                    [P, nkb, P], FP8, tag=f"wt8{j}", name="wt8", bufs=1
                )
                nc.vector.tensor_copy(wT[:], wTb[:])
                return st, wT

            # emission order tucks slot 0 (whose bf16 mini-qproj + exp is the
            # longest dependency chain) mid-QK so nothing pends at the PSUM
            # pool transition; pass A/B still consume longest-first.
            QK_ORDER = [1, 3, 5, 0, 7, 6, 4, 2]
            with tc.tile_pool(name="qkps", bufs=4, space="PSUM") as qkps:
                scored = {jj: emit_scores(jj) for jj in QK_ORDER}
            staged = [(jj, *scored[jj]) for jj in SLOT_ORDER]

            # ---- PV pass A: (w^T already staged by the xbar) @ x8 in fp8
            # DoubleRow (slot 0 bf16) -- pure matmul streaming. (wx)
            # transposes also via xbar; the host packing of WvT matches the
            # d*128+p blocked layout both produce.
            wxT_all = []

            with tc.tile_pool(name="wxps", bufs=6, space="PSUM") as wxps:
                for si, (j, st, wT) in enumerate(staged):
                    nkb = CJ[j] * KCH // P
                    po = [
                        wxps.tile([P, 512], F32, tag="wx", name=f"wx{ec}")
                        for ec in range(2)
                    ]
                    if CJ[j] == 1:
                        for kb in range(nkb):
                            for ec in range(2):
                                nc.tensor.matmul(
                                    po[ec][:],
                                    lhsT=wT[:, kb, :],
                                    rhs=xn16[:, kb, ec * 512 : (ec + 1) * 512],
                                    start=(kb == 0),
                                    stop=(kb == nkb - 1),
                                )
                    else:
                        npair = nkb // 2
                        for kbp in range(npair):
                            for ec in range(2):
                                nc.tensor.matmul(
                                    po[ec][:],
                                    lhsT=wT[:, 2 * kbp : 2 * kbp + 2, :],
                                    rhs=xnt[kbp // 2][
                                        :,
                                        (2 * kbp) % 4 : (2 * kbp) % 4 + 2,
                                        ec * 512 : (ec + 1) * 512,
                                    ],
                                    perf_mode=DR,
                                    start=(kbp == 0),
                                    stop=(kbp == npair - 1),
                                )
                    wx_sb = apool.tile(
                        [P, E], BF16, tag="wx", name="wx_sb", bufs=3
                    )
                    nc.scalar.copy(wx_sb[:, 0:512], po[0][:])
                    nc.vector.tensor_copy(wx_sb[:, 512:1024], po[1][:])
                    wxT = wxtpool.tile([P, DT, P], BF16, tag="wxT", name="wxT")
                    nc.sync.dma_start(wxT[:], wx_sb[:], transpose=True)
                    wxT_all.append(wxT)

            # ---- PV pass B: (wx)^T @ Wv^T, scaled by 1/sum, DMA out.
            with tc.tile_pool(name="pvps", bufs=4, space="PSUM") as pvps:
                for si, (j, st, _) in enumerate(staged):
                    wxT = wxT_all[si]
                    po = [
                        pvps.tile([P, 512], F32, tag="pv", name=f"po{ec}")
                        for ec in range(2)
                    ]
                    for d in range(DT):
                        for ec in range(2):
                            nc.tensor.matmul(
                                po[ec][:],
                                lhsT=wxT[:, d, :],
                                rhs=wvt[d // 2][:, d % 2, ec * 512 : (ec + 1) * 512],
                                start=(d == 0),
                                stop=(d == DT - 1),
                            )
                    ot = apool.tile([P, E], F32, tag="out", name="ot")
                    nc.scalar.mul(ot[:, 0:512], po[0][:], st[:, 7:8])
                    nc.vector.tensor_scalar_mul(
                        ot[:, 512:1024], po[1][:], st[:, 7:8]
                    )
                    nc.sync.dma_start(out_d[j * P : (j + 1) * P, :], ot[:])

    nc.compile()
    return nc


_NC_CACHE = None


def _get_nc():
    global _NC_CACHE
    if _NC_CACHE is None:
        _NC_CACHE = build_kernel()
    return _NC_CACHE


def _pack_inputs(x, Wq, Wk, Wv):
    """Host-side relayout + weight folding."""
    bf = ml_dtypes.bfloat16
    f8 = ml_dtypes.float8_e4m3

    def to8(a):
        return np.clip(a, -240.0, 240.0).astype(f8)

    # folded scores matrix: scores = xq @ M @ xk^T with M = 32*(Wq^T @ Wk)
    # (the 32 re-centers fp8 quantization; the exp scale absorbs it).
    # packed for the q-projection lhsT: mp[p, jt, d, el] = M[d*128+p, jt*128+el]
    M32 = 32.0 * (
        Wq.T.astype(np.float64) @ Wk.astype(np.float64)
    ).astype(np.float32)
    m4 = M32.reshape(DT, P, DT, P).transpose(1, 2, 0, 3)
    mp = np.ascontiguousarray(to8(m4))
    mp16 = np.ascontiguousarray(m4.astype(bf))
    # Wv packed d-outer to match the xbar-transposed (wx)^T layout:
    # wxT[p, d, q] = wx[q, d*128+p], so [p, d, e] = Wv[e, d*128+p]
    wvp = np.ascontiguousarray(
        Wv.reshape(E, DT, P).transpose(2, 1, 0).astype(bf)
    )

    # causal masks per slot (identical formula for both cores' block lists)
    def packmask(blocks):
        m = np.zeros((NSLOT, P, KCH), np.float32)
        for j, blk in enumerate(blocks):
            cc = np.arange(KCH)[None, :] + (CJ[j] - 1) * KCH  # key col
            rr = np.arange(P)[:, None] + blk * P              # query row
            m[j] = np.where(cc <= rr, 0.0, MASK_VAL)
        return np.ascontiguousarray(m.transpose(1, 0, 2).astype(bf))  # [P,slot,KCH]

    masks = [packmask(QBLOCKS[0]), packmask(QBLOCKS[1])]

    in_maps = []
    for c in range(N_CORES):
        b, h = divmod(c, 2)
        xb = x[b]  # [S, D]
        xt = np.ascontiguousarray(
            to8(xb).reshape(S, DT, P).transpose(2, 1, 0)
        )
        xnat = np.ascontiguousarray(
            to8(xb).reshape(S // P, P, D).transpose(1, 0, 2)
        )
        xn16 = np.ascontiguousarray(
            xb[: 2 * P].reshape(2, P, D).transpose(1, 0, 2).astype(bf)
        )
        xt16 = np.ascontiguousarray(
            xb[:KCH].reshape(KCH, DT, P).transpose(2, 1, 0).astype(bf)
        )
        rows = np.concatenate(
            [np.arange(blk * P, (blk + 1) * P) for blk in QBLOCKS[h]]
        )
        xq = xb[rows]  # [SQ, D]
        xqt = np.ascontiguousarray(
            to8(xq).reshape(SQ, DT, P).transpose(2, 1, 0)
        )
        xq16 = np.ascontiguousarray(
            xq[:P].reshape(P, DT, P).transpose(2, 1, 0).astype(bf)
        )
        in_maps.append(
            {
                "xT": xt,
                "xn": xnat,
                "xqT": xqt,
                "MT": mp,
                "WvT": wvp,
                "masks": masks[h],
                "MT16": mp16,
                "xq16": xq16,
                "xT16": xt16,
                "xn16": xn16,
            }
        )
    return in_maps


def kernel(x, Wq, Wk, Wv, _spmd_kwargs=None, _results_out=None):
    x = np.asarray(x, dtype=np.float32)
    Wq = np.asarray(Wq, dtype=np.float32)
    Wk = np.asarray(Wk, dtype=np.float32)
    Wv = np.asarray(Wv, dtype=np.float32)
    assert x.shape == (B, S, D)

    nc = _get_nc()
    in_maps = _pack_inputs(x, Wq, Wk, Wv)
    res = run_bass_kernel_spmd(
        nc, in_maps, list(range(N_CORES)), **(_spmd_kwargs or {})
    )
    if _results_out is not None:
        _results_out.append(res)

    out = np.empty((B, S, E), np.float32)
    for c in range(N_CORES):
        b, h = divmod(c, 2)
        o = res.results[c]["out"]
        for j, blk in enumerate(QBLOCKS[h]):
            out[b, blk * P : (blk + 1) * P, :] = o[j * P : (j + 1) * P, :]
    return out


# revision 33
# speedup vs baseline: 1.1243x; 1.1243x over previous
"""Trainium2 Bass kernel for single-head causal attention.

Problem: x[4,2048,1024] f32; Wq/Wk/Wv [1024,1024] (torch Linear layout, y = x@W.T).
  q,k,v = x@W.T ; scores = q@k.T (causal masked, scaled 1/sqrt(1024)) ;
  out = softmax(scores)@v.

Weight folding: scores = xq (Wq^T Wk) xk^T, so with M := 32*(Wq^T Wk)
precomputed on the host the K projection disappears -- x^T itself is the key
matrix. Likewise out = w @ x @ Wv^T, so the V projection collapses to a
per-slot (w.x) @ Wv^T postmultiply.

fp8 everywhere the error averages out: the softmax temperature (1/32) makes
the score path error-tolerant, and PV pass A's quantization noise is iid
across keys so it averages down by sqrt(n_keys). Both run fp8-e4m3 with
DoubleRow perf mode (2 k-tiles per pass, ~1.5x bf16). Pass B (contraction
over d with fresh noise per d: no averaging) stays bf16.

Early rows have few keys -> no averaging, so slot 0 (each core's first query
block = global rows 0..255, 256-key causal extent) runs its whole score +
pass-A path in bf16: a small bf16 q-projection for its 128 queries, bf16
keys, bf16 pass A. This drops max-rel-err from ~1.4e-2 to ~4e-3 (gate 2e-2).

Softmax drops max-subtraction: scores'/1024 = scores/32 is bounded ~1.8 so
exp can't overflow; masked entries underflow to 0; exp runs per-chunk
straight from PSUM with accumulated partial sums.

Sharding: 2 cores per batch, zig-zag query blocks (identical causal extents
[1,8,2,7,3,6,4,5] chunks of 256 on every core -> one SPMD program).

Per-core pipeline (fp32 PSUM accumulation):
  1. qMT = (xq @ M)^T in fp8 DoubleRow; qMT stored fp8.
  2. QK (slots longest-first) fp8 DoubleRow; mask in-place on PSUM edge
     chunk; exp from PSUM on ACT with accumulated sums; weights stored fp8
     (slot 0: bf16 mini-qproj + bf16 QK, weights bf16).
  3. PV pass A fp8 DoubleRow: fp8 PE-transposes of weight block pairs +
     (w @ x8) accumulation, prev slot's (wx) transposes interleaved.
  4. PV pass B bf16: (wx)^T @ Wv^T, 1/sum fused into PSUM->SBUF, DMA out.
"""

from contextlib import ExitStack

import ml_dtypes
import numpy as np

import concourse.mybir as mybir
import concourse.tile as tile
from concourse import bacc
from concourse.bass_utils import run_bass_kernel_spmd

B, S, D, E = 4, 2048, 1024, 1024
P = 128
N_CORES = 8
DT = D // P          # 8 d-tiles (contraction)
DP = DT // 2         # 4 d-tile PAIRS (fp8 DoubleRow contracts 2 tiles/pass)
SQ = S // 2          # 1024 query rows per core
KCH = 256            # causal-length granularity (key chunk)
NSLOT = SQ // P      # 8 query slots per core

QCH = [512, 512]     # xqT chunking (DoubleRow wants N>=512 passes)
assert sum(QCH) == SQ

# zig-zag query-block assignment: both cores' slots have identical causal
# chunk counts CJ, so one SPMD program serves all cores.
QBLOCKS = [[0, 15, 2, 13, 4, 11, 6, 9], [1, 14, 3, 12, 5, 10, 7, 8]]
CJ = [(b + 1 + 1) // 2 for b in QBLOCKS[0]]  # [1,8,2,7,3,6,4,5]
assert CJ == [(b + 1 + 1) // 2 for b in QBLOCKS[1]]
SLOT_ORDER = sorted(range(NSLOT), key=lambda j: -CJ[j])  # longest first

F32 = mybir.dt.float32
BF16 = mybir.dt.bfloat16
FP8 = mybir.dt.float8e4
DR = mybir.MatmulPerfMode.DoubleRow
AX = mybir.AxisListType.X
EXP = mybir.ActivationFunctionType.Exp
EXP_SCALE = 1.0 / 1024.0   # (1/32 softmax temp) * (1/32 host M-scale)
MASK_VAL = -1.0e9
WPIECES = [(0, 1), (1, 2), (2, 4), (4, 8)]  # M DMA split over out-tiles


def build_kernel():
    nc = bacc.Bacc(
        "TRN2",
        target_bir_lowering=False,
        debug=False,
        num_devices=N_CORES,
        dynamic_dma_scratch_size=64,
    )
    xT_d = nc.dram_tensor("xT", [P, DT, S], FP8, kind="ExternalInput")
    xn_d = nc.dram_tensor("xn", [P, S // P, D], FP8, kind="ExternalInput")
    xqT_d = nc.dram_tensor("xqT", [P, DT, SQ], FP8, kind="ExternalInput")
    m_d = nc.dram_tensor("MT", [P, DT, DT, P], FP8, kind="ExternalInput")
    wv_d = nc.dram_tensor("WvT", [P, DT, E], BF16, kind="ExternalInput")
    msk_d = nc.dram_tensor("masks", [P, NSLOT, KCH], BF16, kind="ExternalInput")
    # bf16 sidecar for slot 0 (rows 0..255): M, first 128 gathered queries,
    # first 256 keys (transposed + natural)
    m16_d = nc.dram_tensor("MT16", [P, DT, DT, P], BF16, kind="ExternalInput")
    xq16_d = nc.dram_tensor("xq16", [P, DT, P], BF16, kind="ExternalInput")
    xT16_d = nc.dram_tensor("xT16", [P, DT, KCH], BF16, kind="ExternalInput")
    xn16_d = nc.dram_tensor("xn16", [P, 2, D], BF16, kind="ExternalInput")
    out_d = nc.dram_tensor("out", [SQ, E], F32, kind="ExternalOutput")

    with tile.TileContext(nc) as tc, ExitStack() as ctx:
        # persistent tensors (right side). Bulk inputs are split into tiles
        # of <=4KB/partition so every dma_start is a single segment with its
        # own semaphore -- larger DMAs get chopped into semaphore-chained
        # segments that occupy the issuing queue until the transfer lands,
        # head-of-line blocking everything behind them (the xbar transposes).
        kqv = ctx.enter_context(tc.tile_pool(name="kqv", bufs=1, side="right"))
        # keys x^T by DoubleRow d-pair: xTt[dp][p, s, k] = x[k, (2dp+s)*128+p]
        xTt = [kqv.tile([P, 2, S], FP8, tag=f"xT{dp}", name=f"xT{dp}") for dp in range(DP)]
        # x natural by key-block group: xnt[a][p, b, d] = x[(4a+b)*128+p, d]
        xnt = [kqv.tile([P, 4, D], FP8, tag=f"xn{a}", name=f"xn{a}") for a in range(4)]
        qMT = kqv.tile([P, DT, SQ], FP8, tag="qMT")      # (xq M)^T (fp8)
        # WvT by d-pair: wvt[h][p, s, e] = Wv[e, (2h+s)*128+p]
        wvt = [kqv.tile([P, 2, E], BF16, tag=f"wv{h}", name=f"wv{h}") for h in range(DP)]
        msk = kqv.tile([P, NSLOT, KCH], BF16, tag="msk")
        # M16 by jt-pair: m16t[g][p, s, d, el] = M[d*128+p, (2g+s)*128+el]
        m16t = [kqv.tile([P, 2, DT, P], BF16, tag=f"m16{g}", name=f"m16{g}") for g in range(DP)]
        xq16 = kqv.tile([P, DT, P], BF16, tag="xq16")
        xT16 = kqv.tile([P, DT, KCH], BF16, tag="xT16")
        xn16 = kqv.tile([P, 2, D], BF16, tag="xn16")
        qMT16 = kqv.tile([P, DT, P], BF16, tag="qMT16")

        # ---------------- folded q projection ----------------
        with (
            tc.tile_pool(name="wpool", bufs=1) as wpool,
            tc.tile_pool(name="xpool", bufs=2) as xpool,
            tc.tile_pool(name="pps", bufs=6, space="PSUM") as pps,
        ):
            # HAM warm-up: dummy matmuls on a zeroed tile fill the DMA-init
            # dead zone and un-throttle the PE clock before real work
            warm = xpool.tile([P, 512], BF16, tag="warm", name="warm", bufs=1)
            nc.gpsimd.memset(warm[:], 0.0)
            wps = pps.tile([P, 512], F32, tag="wps", name="wps", bufs=1)
            for _ in range(10):
                nc.tensor.matmul(
                    wps[:], lhsT=warm[:, 0:P], rhs=warm[:], start=True, stop=True
                )
            for _ in range(6):
                nc.tensor.matmul(
                    wps[:, 0:256],
                    lhsT=warm[:, 0:P],
                    rhs=warm[:, 0:256],
                    start=True,
                    stop=True,
                )

            m_sb = wpool.tile([P, DT, DT, P], FP8, tag="M", name="m_sb")
            lo, hi = WPIECES[0]
            nc.sync.dma_start(m_sb[:, lo:hi], m_d[:, lo:hi])
            xqc = []
            t0 = 0
            for ci, csz in enumerate(QCH):
                xc = xpool.tile([P, DT, 512], FP8, tag="x", name="xc")
                nc.sync.dma_start(xc[:, :, 0:csz], xqT_d[:, :, t0 : t0 + csz])
                xqc.append(xc)
                t0 += csz
                if ci == 0:
                    for lo, hi in WPIECES[1:]:
                        nc.sync.dma_start(m_sb[:, lo:hi], m_d[:, lo:hi])
            # bulk streaming inputs, ordered by first use in the attention
            # phases: xT (QK), masks, slot-0 bf16 sidecar (mini-qproj sits
            # mid-QK), xn (pass A), WvT (pass B) -- all single-segment DMAs
            for dp in range(DP):
                nc.sync.dma_start(xTt[dp][:], xT_d[:, 2 * dp : 2 * dp + 2])
            nc.sync.dma_start(msk[:], msk_d[:])
            for g in range(DP):
                nc.sync.dma_start(m16t[g][:], m16_d[:, 2 * g : 2 * g + 2])
            nc.sync.dma_start(xq16[:], xq16_d[:])
            nc.sync.dma_start(xT16[:], xT16_d[:])

            t0 = 0
            for ci, csz in enumerate(QCH):
                xc = xqc[ci]
                for j_t in range(DT):
                    ps = pps.tile([P, 512], F32, tag="ps", name="ps")
                    for dp in range(DP):
                        nc.tensor.matmul(
                            ps[:, 0:csz],
                            lhsT=m_sb[:, j_t, 2 * dp : 2 * dp + 2, :],
                            rhs=xc[:, 2 * dp : 2 * dp + 2, 0:csz],
                            perf_mode=DR,
                            start=(dp == 0),
                            stop=(dp == DP - 1),
                        )
                    nc.scalar.copy(qMT[:, j_t, t0 : t0 + csz], ps[:, 0:csz])
                t0 += csz

        # ---------------- attention ----------------
        with (
            tc.tile_pool(name="apool", bufs=2) as apool,
            tc.tile_pool(name="wtpool", bufs=4) as wtpool,
            tc.tile_pool(name="wxtpool", bufs=NSLOT) as wxtpool,
            tc.tile_pool(name="stpool", bufs=NSLOT, side="right") as stpool,
        ):
            def emit_scores(j):
                """QK + mask + exp + sum for slot j; transposes the weight
                tile through the DMA xbar (and quantizes to fp8 on DVE for
                the DoubleRow pass-A lhsT). Slot 0 -> bf16 path."""
                C = CJ[j]
                L = C * KCH
                nkb = C * KCH // P
                st = stpool.tile([P, 8], F32, tag="st", name="st")
                if C == 1:
                    # bf16 mini-qproj for slot-0's 128 queries (queued at the
                    # QK tail: SLOT_ORDER puts this slot last)
                    for j_t in range(DT):
                        ps = qkps.tile([P, 512], F32, tag="qk", name="mq")
                        for d in range(DT):
                            nc.tensor.matmul(
                                ps[:, 0:P],
                                lhsT=m16t[j_t // 2][:, j_t % 2, d, :],
                                rhs=xq16[:, d, :],
                                start=(d == 0),
                                stop=(d == DT - 1),
                            )
                        nc.scalar.copy(qMT16[:, j_t, :], ps[:, 0:P])
                    wts = apool.tile(
                        [P, KCH], BF16, tag="wts16", name="wts16", bufs=1
                    )
                    ps = qkps.tile([P, 512], F32, tag="qk", name="qk")
                    for d in range(DT):
                        nc.tensor.matmul(
                            ps[:, 0:KCH],
                            lhsT=qMT16[:, d, :],
                            rhs=xT16[:, d, :],
                            start=(d == 0),
                            stop=(d == DT - 1),
                        )
                    nc.vector.tensor_add(
                        ps[:, 0:KCH], ps[:, 0:KCH], msk[:, j, :]
                    )
                    nc.scalar.activation(
                        wts[:, 0:KCH],
                        ps[:, 0:KCH],
                        EXP,
                        scale=EXP_SCALE,
                        accum_out=st[:, 0:1],
                    )
                    nc.vector.reciprocal(st[:, 7:8], st[:, 0:1])
                    wT = wtpool.tile(
                        [P, 2, P], BF16, tag="wt16s", name="wt16s", bufs=1
                    )
                    nc.sync.dma_start(wT[:], wts[:, 0:KCH], transpose=True)
                    return st, wT

                groups = [(g * 512, 512) for g in range(C // 2)]
                if C % 2:
                    groups.append(((C // 2) * 512, 256))
                wts = apool.tile(
                    [P, S], BF16, tag="wts8", name="wts8", bufs=NSLOT - 1
                )
                nch = len(groups)
                for ci, (k0, ksz) in enumerate(groups):
                    ps = qkps.tile([P, 512], F32, tag="qk", name="qk")
                    for dp in range(DP):
                        nc.tensor.matmul(
                            ps[:, 0:ksz],
                            lhsT=qMT[:, 2 * dp : 2 * dp + 2, j * P : (j + 1) * P],
                            rhs=xTt[dp][:, :, k0 : k0 + ksz],
                            perf_mode=DR,
                            start=(dp == 0),
                            stop=(dp == DP - 1),
                        )
                    if k0 + ksz == L:
                        # causal edge: host mask covers the last 256 keys
                        nc.vector.tensor_add(
                            ps[:, ksz - 256 : ksz],
                            ps[:, ksz - 256 : ksz],
                            msk[:, j, :],
                        )
                    nc.scalar.activation(
                        wts[:, k0 : k0 + ksz],
                        ps[:, 0:ksz],
                        EXP,
                        scale=EXP_SCALE,
                        accum_out=st[:, ci : ci + 1],
                    )
                if nch > 1:
                    nc.vector.tensor_reduce(
                        st[:, 6:7], st[:, 0:nch], axis=AX, op=mybir.AluOpType.add
                    )
                    nc.vector.reciprocal(st[:, 7:8], st[:, 6:7])
                else:
                    nc.vector.reciprocal(st[:, 7:8], st[:, 0:1])
                wTb = wtpool.tile(
                    [P, nkb, P], BF16, tag=f"wtb{j}", name="wtb", bufs=1
                )
                nc.sync.dma_start(wTb[:], wts[:, 0:L], transpose=True)
                wT = wtpool.tile(
                    [P, nkb, P], FP8, tag=f"wt8{j}", name="wt8", bufs=1
                )
                nc.vector.tensor_copy(wT[:], wTb[:])
                return st, wT

            # emission order tucks slot 0 (whose bf16 mini-qproj + exp is the
            # longest dependency chain) mid-QK so nothing pends at the PSUM
            # pool transition; pass A/B still consume longest-first.
            QK_ORDER = [1, 3, 5, 0, 7, 6, 4, 2]
            with tc.tile_pool(name="qkps", bufs=4, space="PSUM") as qkps:
                scored = {}
                for qi, jj in enumerate(QK_ORDER):
                    scored[jj] = emit_scores(jj)
                    if qi == 3:
                        # pass A/B inputs issue only now: their chained DMA
                        # segments occupy the sync queue until the transfers
                        # land, and would head-of-line block the big slots'
                        # weight transposes pass A needs first. The last
                        # small slots' transposes queue behind these chains
                        # but aren't consumed until late in pass A.
                        for a in range(4):
                            nc.sync.dma_start(
                                xnt[a][:], xn_d[:, 4 * a : 4 * a + 4]
                            )
                        nc.sync.dma_start(xn16[:], xn16_d[:])
                        for h in range(DP):
                            nc.sync.dma_start(
                                wvt[h][:], wv_d[:, 2 * h : 2 * h + 2]
                            )
            staged = [(jj, *scored[jj]) for jj in SLOT_ORDER]

            # ---- PV pass A: (w^T already staged by the xbar) @ x8 in fp8
            # DoubleRow (slot 0 bf16) -- pure matmul streaming. (wx)
            # transposes also via xbar; the host packing of WvT matches the
            # d*128+p blocked layout both produce.
            wxT_all = []

            with tc.tile_pool(name="wxps", bufs=6, space="PSUM") as wxps:
                for si, (j, st, wT) in enumerate(staged):
                    nkb = CJ[j] * KCH // P
                    po = [
                        wxps.tile([P, 512], F32, tag="wx", name=f"wx{ec}")
                        for ec in range(2)
                    ]
                    if CJ[j] == 1:
                        for kb in range(nkb):
                            for ec in range(2):
                                nc.tensor.matmul(
                                    po[ec][:],
                                    lhsT=wT[:, kb, :],
                                    rhs=xn16[:, kb, ec * 512 : (ec + 1) * 512],
                                    start=(kb == 0),
                                    stop=(kb == nkb - 1),
                                )
                    else:
                        npair = nkb // 2
                        for kbp in range(npair):
                            for ec in range(2):
                                nc.tensor.matmul(
                                    po[ec][:],
                                    lhsT=wT[:, 2 * kbp : 2 * kbp + 2, :],
                                    rhs=xnt[kbp // 2][
                                        :,
                                        (2 * kbp) % 4 : (2 * kbp) % 4 + 2,
                                        ec * 512 : (ec + 1) * 512,
                                    ],
                                    perf_mode=DR,
                                    start=(kbp == 0),
                                    stop=(kbp == npair - 1),
                                )
                    wx_sb = apool.tile(
                        [P, E], BF16, tag="wx", name="wx_sb", bufs=3
                    )
                    nc.scalar.copy(wx_sb[:, 0:512], po[0][:])
                    nc.vector.tensor_copy(wx_sb[:, 512:1024], po[1][:])
                    wxT = wxtpool.tile([P, DT, P], BF16, tag="wxT", name="wxT")
                    nc.sync.dma_start(wxT[:], wx_sb[:], transpose=True)
                    wxT_all.append(wxT)

            # ---- PV pass B: (wx)^T @ Wv^T, scaled by 1/sum, DMA out.
            with tc.tile_pool(name="pvps", bufs=4, space="PSUM") as pvps:
                for si, (j, st, _) in enumerate(staged):
                    wxT = wxT_all[si]
                    po = [
                        pvps.tile([P, 512], F32, tag="pv", name=f"po{ec}")
                        for ec in range(2)
                    ]
                    for d in range(DT):
                        for ec in range(2):
                            nc.tensor.matmul(
                                po[ec][:],
                                lhsT=wxT[:, d, :],
                                rhs=wvt[d // 2][:, d % 2, ec * 512 : (ec + 1) * 512],
                                start=(d == 0),
                                stop=(d == DT - 1),
                            )
                    ot = apool.tile([P, E], F32, tag="out", name="ot")
                    nc.scalar.mul(ot[:, 0:512], po[0][:], st[:, 7:8])
                    nc.vector.tensor_scalar_mul(
                        ot[:, 512:1024], po[1][:], st[:, 7:8]
                    )
                    nc.sync.dma_start(out_d[j * P : (j + 1) * P, :], ot[:])

    nc.compile()
    return nc


_NC_CACHE = None


def _get_nc():
    global _NC_CACHE
    if _NC_CACHE is None:
        _NC_CACHE = build_kernel()
    return _NC_CACHE


def _pack_inputs(x, Wq, Wk, Wv):
    """Host-side relayout + weight folding."""
    bf = ml_dtypes.bfloat16
    f8 = ml_dtypes.float8_e4m3

    def to8(a):
        return np.clip(a, -240.0, 240.0).astype(f8)

    # folded scores matrix: scores = xq @ M @ xk^T with M = 32*(Wq^T @ Wk)
    # (the 32 re-centers fp8 quantization; the exp scale absorbs it).
    # packed for the q-projection lhsT: mp[p, jt, d, el] = M[d*128+p, jt*128+el]
    M32 = 32.0 * (
        Wq.T.astype(np.float64) @ Wk.astype(np.float64)
    ).astype(np.float32)
    m4 = M32.reshape(DT, P, DT, P).transpose(1, 2, 0, 3)
    mp = np.ascontiguousarray(to8(m4))
    mp16 = np.ascontiguousarray(m4.astype(bf))
    # Wv packed d-outer to match the xbar-transposed (wx)^T layout:
    # wxT[p, d, q] = wx[q, d*128+p], so [p, d, e] = Wv[e, d*128+p]
    wvp = np.ascontiguousarray(
        Wv.reshape(E, DT, P).transpose(2, 1, 0).astype(bf)
    )

    # causal masks per slot (identical formula for both cores' block lists)
    def packmask(blocks):
        m = np.zeros((NSLOT, P, KCH), np.float32)
        for j, blk in enumerate(blocks):
            cc = np.arange(KCH)[None, :] + (CJ[j] - 1) * KCH  # key col
            rr = np.arange(P)[:, None] + blk * P              # query row
            m[j] = np.where(cc <= rr, 0.0, MASK_VAL)
        return np.ascontiguousarray(m.transpose(1, 0, 2).astype(bf))  # [P,slot,KCH]

    masks = [packmask(QBLOCKS[0]), packmask(QBLOCKS[1])]

    in_maps = []
    for c in range(N_CORES):
        b, h = divmod(c, 2)
        xb = x[b]  # [S, D]
        xt = np.ascontiguousarray(
            to8(xb).reshape(S, DT, P).transpose(2, 1, 0)
        )
        xnat = np.ascontiguousarray(
            to8(xb).reshape(S // P, P, D).transpose(1, 0, 2)
        )
        xn16 = np.ascontiguousarray(
            xb[: 2 * P].reshape(2, P, D).transpose(1, 0, 2).astype(bf)
        )
        xt16 = np.ascontiguousarray(
            xb[:KCH].reshape(KCH, DT, P).transpose(2, 1, 0).astype(bf)
        )
        rows = np.concatenate(
            [np.arange(blk * P, (blk + 1) * P) for blk in QBLOCKS[h]]
        )
        xq = xb[rows]  # [SQ, D]
        xqt = np.ascontiguousarray(
            to8(xq).reshape(SQ, DT, P).transpose(2, 1, 0)
        )
        xq16 = np.ascontiguousarray(
            xq[:P].reshape(P, DT, P).transpose(2, 1, 0).astype(bf)
        )
        in_maps.append(
            {
                "xT": xt,
                "xn": xnat,
                "xqT": xqt,
                "MT": mp,
                "WvT": wvp,
                "masks": masks[h],
                "MT16": mp16,
                "xq16": xq16,
                "xT16": xt16,
                "xn16": xn16,
            }
        )
    return in_maps


def kernel(x, Wq, Wk, Wv, _spmd_kwargs=None, _results_out=None):
    x = np.asarray(x, dtype=np.float32)
    Wq = np.asarray(Wq, dtype=np.float32)
    Wk = np.asarray(Wk, dtype=np.float32)
    Wv = np.asarray(Wv, dtype=np.float32)
    assert x.shape == (B, S, D)

    nc = _get_nc()
    in_maps = _pack_inputs(x, Wq, Wk, Wv)
    res = run_bass_kernel_spmd(
        nc, in_maps, list(range(N_CORES)), **(_spmd_kwargs or {})
    )
    if _results_out is not None:
        _results_out.append(res)

    out = np.empty((B, S, E), np.float32)
    for c in range(N_CORES):
        b, h = divmod(c, 2)
        o = res.results[c]["out"]
        for j, blk in enumerate(QBLOCKS[h]):
            out[b, blk * P : (blk + 1) * P, :] = o[j * P : (j + 1) * P, :]
    return out


# revision 36
# speedup vs baseline: 1.2081x; 1.0745x over previous
"""Trainium2 Bass kernel for single-head causal attention.

Problem: x[4,2048,1024] f32; Wq/Wk/Wv [1024,1024] (torch Linear layout, y = x@W.T).
  q,k,v = x@W.T ; scores = q@k.T (causal masked, scaled 1/sqrt(1024)) ;
  out = softmax(scores)@v.

Weight folding: scores = xq (Wq^T Wk) xk^T, so with M := 32*(Wq^T Wk)
precomputed on the host the K projection disappears -- x^T itself is the key
matrix. Likewise out = w @ x @ Wv^T, so the V projection collapses to a
per-slot (w.x) @ Wv^T postmultiply.

fp8 everywhere the error averages out: the softmax temperature (1/32) makes
the score path error-tolerant, and PV pass A's quantization noise is iid
across keys so it averages down by sqrt(n_keys). Both run fp8-e4m3 with
DoubleRow perf mode (2 k-tiles per pass, ~1.5x bf16). Pass B (contraction
over d with fresh noise per d: no averaging) stays bf16.

Early rows have few keys -> no averaging, so slot 0 (each core's first query
block = global rows 0..255, 256-key causal extent) runs its whole score +
pass-A path in bf16: a small bf16 q-projection for its 128 queries, bf16
keys, bf16 pass A. This drops max-rel-err from ~1.4e-2 to ~4e-3 (gate 2e-2).

Softmax drops max-subtraction: scores'/1024 = scores/32 is bounded ~1.8 so
exp can't overflow; masked entries underflow to 0; exp runs per-chunk
straight from PSUM with accumulated partial sums.

Sharding: 2 cores per batch, zig-zag query blocks (identical causal extents
[1,8,2,7,3,6,4,5] chunks of 256 on every core -> one SPMD program).

Per-core pipeline (fp32 PSUM accumulation):
  1. qMT = (xq @ M)^T in fp8 DoubleRow; qMT stored fp8.
  2. QK (slots longest-first) fp8 DoubleRow; mask in-place on PSUM edge
     chunk; exp from PSUM on ACT with accumulated sums; weights stored fp8
     (slot 0: bf16 mini-qproj + bf16 QK, weights bf16).
  3. PV pass A fp8 DoubleRow: fp8 PE-transposes of weight block pairs +
     (w @ x8) accumulation, prev slot's (wx) transposes interleaved.
  4. PV pass B bf16: (wx)^T @ Wv^T, 1/sum fused into PSUM->SBUF, DMA out.
"""

from contextlib import ExitStack

import ml_dtypes
import numpy as np

import concourse.mybir as mybir
import concourse.tile as tile
from concourse import bacc
from concourse.bass_utils import run_bass_kernel_spmd
from concourse.masks import make_identity

B, S, D, E = 4, 2048, 1024, 1024
P = 128
N_CORES = 8
DT = D // P          # 8 d-tiles (contraction)
DP = DT // 2         # 4 d-tile PAIRS (fp8 DoubleRow contracts 2 tiles/pass)
SQ = S // 2          # 1024 query rows per core
KCH = 256            # causal-length granularity (key chunk)
NSLOT = SQ // P      # 8 query slots per core

QCH = [512, 512]     # xqT chunking (DoubleRow wants N>=512 passes)
assert sum(QCH) == SQ

# zig-zag query-block assignment: both cores' slots have identical causal
# chunk counts CJ, so one SPMD program serves all cores.
QBLOCKS = [[0, 15, 2, 13, 4, 11, 6, 9], [1, 14, 3, 12, 5, 10, 7, 8]]
CJ = [(b + 1 + 1) // 2 for b in QBLOCKS[0]]  # [1,8,2,7,3,6,4,5]
assert CJ == [(b + 1 + 1) // 2 for b in QBLOCKS[1]]
SLOT_ORDER = sorted(range(NSLOT), key=lambda j: -CJ[j])  # longest first

F32 = mybir.dt.float32
BF16 = mybir.dt.bfloat16
FP8 = mybir.dt.float8e4
DR = mybir.MatmulPerfMode.DoubleRow
AX = mybir.AxisListType.X
EXP = mybir.ActivationFunctionType.Exp
EXP_SCALE = 1.0 / 1024.0   # (1/32 softmax temp) * (1/32 host M-scale)
MASK_VAL = -1.0e9
WPIECES = [(0, 1), (1, 2), (2, 4), (4, 8)]  # M DMA split over out-tiles


def build_kernel():
    nc = bacc.Bacc(
        "TRN2",
        target_bir_lowering=False,
        debug=False,
        num_devices=N_CORES,
        dynamic_dma_scratch_size=64,
    )
    xT_d = nc.dram_tensor("xT", [P, DT, S], FP8, kind="ExternalInput")
    xn_d = nc.dram_tensor("xn", [P, S // P, D], FP8, kind="ExternalInput")
    xqT_d = nc.dram_tensor("xqT", [P, DT, SQ], FP8, kind="ExternalInput")
    m_d = nc.dram_tensor("MT", [P, DT, DT, P], FP8, kind="ExternalInput")
    wv_d = nc.dram_tensor("WvT", [P, DT, E], BF16, kind="ExternalInput")
    msk_d = nc.dram_tensor("masks", [P, NSLOT, KCH], BF16, kind="ExternalInput")
    # bf16 sidecar for slot 0 (rows 0..255): M, first 128 gathered queries,
    # first 256 keys (transposed + natural)
    m16_d = nc.dram_tensor("MT16", [P, DT, DT, P], BF16, kind="ExternalInput")
    xq16_d = nc.dram_tensor("xq16", [P, DT, P], BF16, kind="ExternalInput")
    xT16_d = nc.dram_tensor("xT16", [P, DT, KCH], BF16, kind="ExternalInput")
    xn16_d = nc.dram_tensor("xn16", [P, 2, D], BF16, kind="ExternalInput")
    out_d = nc.dram_tensor("out", [SQ, E], F32, kind="ExternalOutput")

    with tile.TileContext(nc) as tc, ExitStack() as ctx:
        # persistent tensors (right side). Bulk inputs are split into tiles
        # of <=4KB/partition so every dma_start is a single segment with its
        # own semaphore -- larger DMAs get chopped into semaphore-chained
        # segments that occupy the issuing queue until the transfer lands,
        # head-of-line blocking everything behind them (the xbar transposes).
        kqv = ctx.enter_context(tc.tile_pool(name="kqv", bufs=1, side="right"))
        # keys x^T by DoubleRow d-pair: xTt[dp][p, s, k] = x[k, (2dp+s)*128+p]
        xTt = [kqv.tile([P, 2, S], FP8, tag=f"xT{dp}", name=f"xT{dp}") for dp in range(DP)]
        # x natural by key-block group: xnt[a][p, b, d] = x[(4a+b)*128+p, d]
        xnt = [kqv.tile([P, 4, D], FP8, tag=f"xn{a}", name=f"xn{a}") for a in range(4)]
        qMT = kqv.tile([P, DT, SQ], FP8, tag="qMT")      # (xq M)^T (fp8)
        # WvT by d-pair: wvt[h][p, s, e] = Wv[e, (2h+s)*128+p]
        wvt = [kqv.tile([P, 2, E], BF16, tag=f"wv{h}", name=f"wv{h}") for h in range(DP)]
        msk = kqv.tile([P, NSLOT, KCH], BF16, tag="msk")
        # M16 by jt-pair: m16t[g][p, s, d, el] = M[d*128+p, (2g+s)*128+el]
        m16t = [kqv.tile([P, 2, DT, P], BF16, tag=f"m16{g}", name=f"m16{g}") for g in range(DP)]
        xq16 = kqv.tile([P, DT, P], BF16, tag="xq16")
        xT16 = kqv.tile([P, DT, KCH], BF16, tag="xT16")
        xn16 = kqv.tile([P, 2, D], BF16, tag="xn16")
        qMT16 = kqv.tile([P, DT, P], BF16, tag="qMT16")

        # ---------------- folded q projection ----------------
        with (
            tc.tile_pool(name="wpool", bufs=1) as wpool,
            tc.tile_pool(name="xpool", bufs=2) as xpool,
            tc.tile_pool(name="pps", bufs=6, space="PSUM") as pps,
        ):
            # HAM warm-up: dummy matmuls on a zeroed tile fill the DMA-init
            # dead zone and un-throttle the PE clock before real work
            warm = xpool.tile([P, 512], BF16, tag="warm", name="warm", bufs=1)
            nc.gpsimd.memset(warm[:], 0.0)
            wps = pps.tile([P, 512], F32, tag="wps", name="wps", bufs=1)
            for _ in range(10):
                nc.tensor.matmul(
                    wps[:], lhsT=warm[:, 0:P], rhs=warm[:], start=True, stop=True
                )
            for _ in range(6):
                nc.tensor.matmul(
                    wps[:, 0:256],
                    lhsT=warm[:, 0:P],
                    rhs=warm[:, 0:256],
                    start=True,
                    stop=True,
                )

            m_sb = wpool.tile([P, DT, DT, P], FP8, tag="M", name="m_sb")
            lo, hi = WPIECES[0]
            nc.sync.dma_start(m_sb[:, lo:hi], m_d[:, lo:hi])
            xqc = []
            t0 = 0
            for ci, csz in enumerate(QCH):
                xc = xpool.tile([P, DT, 512], FP8, tag="x", name="xc")
                nc.sync.dma_start(xc[:, :, 0:csz], xqT_d[:, :, t0 : t0 + csz])
                xqc.append(xc)
                t0 += csz
                if ci == 0:
                    for lo, hi in WPIECES[1:]:
                        nc.sync.dma_start(m_sb[:, lo:hi], m_d[:, lo:hi])
            # bulk streaming inputs, ordered by first use in the attention
            # phases: xT (QK), masks, slot-0 bf16 sidecar (mini-qproj sits
            # mid-QK), xn (pass A), WvT (pass B) -- all single-segment DMAs
            for dp in range(DP):
                nc.sync.dma_start(xTt[dp][:], xT_d[:, 2 * dp : 2 * dp + 2])
            nc.sync.dma_start(msk[:], msk_d[:])
            for g in range(DP):
                nc.sync.dma_start(m16t[g][:], m16_d[:, 2 * g : 2 * g + 2])
            nc.sync.dma_start(xq16[:], xq16_d[:])
            nc.sync.dma_start(xT16[:], xT16_d[:])
            for a in range(4):
                nc.sync.dma_start(xnt[a][:], xn_d[:, 4 * a : 4 * a + 4])
            nc.sync.dma_start(xn16[:], xn16_d[:])
            for h in range(DP):
                nc.sync.dma_start(wvt[h][:], wv_d[:, 2 * h : 2 * h + 2])

            t0 = 0
            for ci, csz in enumerate(QCH):
                xc = xqc[ci]
                for j_t in range(DT):
                    ps = pps.tile([P, 512], F32, tag="ps", name="ps")
                    for dp in range(DP):
                        nc.tensor.matmul(
                            ps[:, 0:csz],
                            lhsT=m_sb[:, j_t, 2 * dp : 2 * dp + 2, :],
                            rhs=xc[:, 2 * dp : 2 * dp + 2, 0:csz],
                            perf_mode=DR,
                            start=(dp == 0),
                            stop=(dp == DP - 1),
                        )
                    nc.scalar.copy(qMT[:, j_t, t0 : t0 + csz], ps[:, 0:csz])
                t0 += csz

        # ---------------- attention ----------------
        with (
            tc.tile_pool(name="apool", bufs=2) as apool,
            tc.tile_pool(name="wtpool", bufs=4) as wtpool,
            tc.tile_pool(name="wxtpool", bufs=NSLOT) as wxtpool,
            tc.tile_pool(name="stpool", bufs=NSLOT, side="right") as stpool,
            tc.tile_pool(name="c1pool", bufs=1) as c1pool,
        ):
            def emit_scores(j):
                """QK + mask + exp + sum for slot j; transposes the weight
                tile through the DMA xbar (and quantizes to fp8 on DVE for
                the DoubleRow pass-A lhsT). Slot 0 -> bf16 path."""
                C = CJ[j]
                L = C * KCH
                nkb = C * KCH // P
                st = stpool.tile([P, 8], F32, tag="st", name="st")
                if C == 1:
                    # bf16 mini-qproj for slot-0's 128 queries (queued at the
                    # QK tail: SLOT_ORDER puts this slot last)
                    for j_t in range(DT):
                        ps = qkps.tile([P, 512], F32, tag="qk", name="mq")
                        for d in range(DT):
                            nc.tensor.matmul(
                                ps[:, 0:P],
                                lhsT=m16t[j_t // 2][:, j_t % 2, d, :],
                                rhs=xq16[:, d, :],
                                start=(d == 0),
                                stop=(d == DT - 1),
                            )
                        nc.scalar.copy(qMT16[:, j_t, :], ps[:, 0:P])
                    wts = apool.tile(
                        [P, KCH], BF16, tag="wts16", name="wts16", bufs=1
                    )
                    ps = qkps.tile([P, 512], F32, tag="qk", name="qk")
                    for d in range(DT):
                        nc.tensor.matmul(
                            ps[:, 0:KCH],
                            lhsT=qMT16[:, d, :],
                            rhs=xT16[:, d, :],
                            start=(d == 0),
                            stop=(d == DT - 1),
                        )
                    nc.vector.tensor_add(
                        ps[:, 0:KCH], ps[:, 0:KCH], msk[:, j, :]
                    )
                    nc.scalar.activation(
                        wts[:, 0:KCH],
                        ps[:, 0:KCH],
                        EXP,
                        scale=EXP_SCALE,
                        accum_out=st[:, 0:1],
                    )
                    nc.vector.reciprocal(st[:, 7:8], st[:, 0:1])
                    wT = wtpool.tile(
                        [P, 2, P], BF16, tag="wt16s", name="wt16s", bufs=1
                    )
                    nc.sync.dma_start(wT[:], wts[:, 0:KCH], transpose=True)
                    return st, wT

                groups = [(g * 512, 512) for g in range(C // 2)]
                if C % 2:
                    groups.append(((C // 2) * 512, 256))
                wts = apool.tile(
                    [P, S], BF16, tag="wts8", name="wts8", bufs=NSLOT - 1
                )
                nch = len(groups)
                for ci, (k0, ksz) in enumerate(groups):
                    ps = qkps.tile([P, 512], F32, tag="qk", name="qk")
                    for dp in range(DP):
                        nc.tensor.matmul(
                            ps[:, 0:ksz],
                            lhsT=qMT[:, 2 * dp : 2 * dp + 2, j * P : (j + 1) * P],
                            rhs=xTt[dp][:, :, k0 : k0 + ksz],
                            perf_mode=DR,
                            start=(dp == 0),
                            stop=(dp == DP - 1),
                        )
                    if k0 + ksz == L:
                        # causal edge: host mask covers the last 256 keys
                        nc.vector.tensor_add(
                            ps[:, ksz - 256 : ksz],
                            ps[:, ksz - 256 : ksz],
                            msk[:, j, :],
                        )
                    nc.scalar.activation(
                        wts[:, k0 : k0 + ksz],
                        ps[:, 0:ksz],
                        EXP,
                        scale=EXP_SCALE,
                        accum_out=st[:, ci : ci + 1],
                    )
                if nch > 1:
                    nc.vector.tensor_reduce(
                        st[:, 6:7], st[:, 0:nch], axis=AX, op=mybir.AluOpType.add
                    )
                    nc.vector.reciprocal(st[:, 7:8], st[:, 6:7])
                else:
                    nc.vector.reciprocal(st[:, 7:8], st[:, 0:1])
                if j in (1, 3):
                    # first two pass-A slots: their xbar transposes would be
                    # queue-blocked behind the input DMA chains until ~47us;
                    # PE pair-transposes in pass A are cheaper than the stall
                    return st, wts
                wTb = wtpool.tile(
                    [P, nkb, P], BF16, tag=f"wtb{j}", name="wtb", bufs=1
                )
                nc.sync.dma_start(wTb[:], wts[:, 0:L], transpose=True)
                wT = wtpool.tile(
                    [P, nkb, P], FP8, tag=f"wt8{j}", name="wt8", bufs=1
                )
                nc.vector.tensor_copy(wT[:], wTb[:])
                return st, wT

            # emission order tucks slot 0 (whose bf16 mini-qproj + exp is the
            # longest dependency chain) mid-QK so nothing pends at the PSUM
            # pool transition; pass A/B still consume longest-first.
            QK_ORDER = [1, 3, 5, 0, 7, 6, 4, 2]
            with tc.tile_pool(name="qkps", bufs=4, space="PSUM") as qkps:
                scored = {jj: emit_scores(jj) for jj in QK_ORDER}
            staged = [(jj, *scored[jj]) for jj in SLOT_ORDER]

            # ---- PV pass A: w^T @ x8 in fp8 DoubleRow (slot 0 bf16).
            # Slots 1/3 transpose weight-block pairs on the PE here (their
            # xbar DMAs would be queue-blocked); the rest arrive
            # pre-transposed via xbar. (wx) transposes also via xbar; the
            # host packing of WvT matches the d*128+p blocked layout.
            wxT_all = []
            ident = c1pool.tile([P, P], BF16, tag="ident")
            make_identity(nc, ident[:])

            with (
                tc.tile_pool(name="wxps", bufs=6, space="PSUM") as wxps,
                tc.tile_pool(name="trps", bufs=2, space="PSUM") as trps,
            ):
                for si, (j, st, wT) in enumerate(staged):
                    nkb = CJ[j] * KCH // P
                    po = [
                        wxps.tile([P, 512], F32, tag="wx", name=f"wx{ec}")
                        for ec in range(2)
                    ]
                    if CJ[j] == 1:
                        for kb in range(nkb):
                            for ec in range(2):
                                nc.tensor.matmul(
                                    po[ec][:],
                                    lhsT=wT[:, kb, :],
                                    rhs=xn16[:, kb, ec * 512 : (ec + 1) * 512],
                                    start=(kb == 0),
                                    stop=(kb == nkb - 1),
                                )
                    elif j in (1, 3):
                        # wT here is the raw weight tile: PE pair-transposes
                        npair = nkb // 2
                        wTq = []

                        def emit_trp(kbp, wts=wT):
                            wTp = wtpool.tile(
                                [P, 2, P], FP8, tag="wTp", name="wTp", bufs=4
                            )
                            for i in range(2):
                                kb = 2 * kbp + i
                                pt = trps.tile(
                                    [P, P], BF16, tag="tr", name="pt"
                                )
                                nc.tensor.transpose(
                                    pt[:],
                                    wts[:, kb * P : (kb + 1) * P],
                                    ident[:],
                                )
                                nc.vector.tensor_copy(wTp[:, i, :], pt[:])
                            wTq.append(wTp)

                        emit_trp(0)
                        if npair > 1:
                            emit_trp(1)
                        for kbp in range(npair):
                            if kbp + 2 < npair:
                                emit_trp(kbp + 2)
                            for ec in range(2):
                                nc.tensor.matmul(
                                    po[ec][:],
                                    lhsT=wTq[kbp][:, 0:2, :],
                                    rhs=xnt[kbp // 2][
                                        :,
                                        (2 * kbp) % 4 : (2 * kbp) % 4 + 2,
                                        ec * 512 : (ec + 1) * 512,
                                    ],
                                    perf_mode=DR,
                                    start=(kbp == 0),
                                    stop=(kbp == npair - 1),
                                )
                    else:
                        npair = nkb // 2
                        for kbp in range(npair):
                            for ec in range(2):
                                nc.tensor.matmul(
                                    po[ec][:],
                                    lhsT=wT[:, 2 * kbp : 2 * kbp + 2, :],
                                    rhs=xnt[kbp // 2][
                                        :,
                                        (2 * kbp) % 4 : (2 * kbp) % 4 + 2,
                                        ec * 512 : (ec + 1) * 512,
                                    ],
                                    perf_mode=DR,
                                    start=(kbp == 0),
                                    stop=(kbp == npair - 1),
                                )
                    wx_sb = apool.tile(
                        [P, E], BF16, tag="wx", name="wx_sb", bufs=3
                    )
                    nc.scalar.copy(wx_sb[:, 0:512], po[0][:])
                    nc.vector.tensor_copy(wx_sb[:, 512:1024], po[1][:])
                    wxT = wxtpool.tile([P, DT, P], BF16, tag="wxT", name="wxT")
                    nc.sync.dma_start(wxT[:], wx_sb[:], transpose=True)
                    wxT_all.append(wxT)

            # ---- PV pass B: (wx)^T @ Wv^T, scaled by 1/sum, DMA out.
            with tc.tile_pool(name="pvps", bufs=4, space="PSUM") as pvps:
                for si, (j, st, _) in enumerate(staged):
                    wxT = wxT_all[si]
                    po = [
                        pvps.tile([P, 512], F32, tag="pv", name=f"po{ec}")
                        for ec in range(2)
                    ]
                    for d in range(DT):
                        for ec in range(2):
                            nc.tensor.matmul(
                                po[ec][:],
                                lhsT=wxT[:, d, :],
                                rhs=wvt[d // 2][:, d % 2, ec * 512 : (ec + 1) * 512],
                                start=(d == 0),
                                stop=(d == DT - 1),
                            )
                    ot = apool.tile([P, E], F32, tag="out", name="ot")
                    nc.scalar.mul(ot[:, 0:512], po[0][:], st[:, 7:8])
                    nc.vector.tensor_scalar_mul(
                        ot[:, 512:1024], po[1][:], st[:, 7:8]
                    )
                    nc.sync.dma_start(out_d[j * P : (j + 1) * P, :], ot[:])

    nc.compile()
    return nc


_NC_CACHE = None


def _get_nc():
    global _NC_CACHE
    if _NC_CACHE is None:
        _NC_CACHE = build_kernel()
    return _NC_CACHE


def _pack_inputs(x, Wq, Wk, Wv):
    """Host-side relayout + weight folding."""
    bf = ml_dtypes.bfloat16
    f8 = ml_dtypes.float8_e4m3

    def to8(a):
        return np.clip(a, -240.0, 240.0).astype(f8)

    # folded scores matrix: scores = xq @ M @ xk^T with M = 32*(Wq^T @ Wk)
    # (the 32 re-centers fp8 quantization; the exp scale absorbs it).
    # packed for the q-projection lhsT: mp[p, jt, d, el] = M[d*128+p, jt*128+el]
    M32 = 32.0 * (
        Wq.T.astype(np.float64) @ Wk.astype(np.float64)
    ).astype(np.float32)
    m4 = M32.reshape(DT, P, DT, P).transpose(1, 2, 0, 3)
    mp = np.ascontiguousarray(to8(m4))
    mp16 = np.ascontiguousarray(m4.astype(bf))
    # Wv packed d-outer to match the xbar-transposed (wx)^T layout:
    # wxT[p, d, q] = wx[q, d*128+p], so [p, d, e] = Wv[e, d*128+p]
    wvp = np.ascontiguousarray(
        Wv.reshape(E, DT, P).transpose(2, 1, 0).astype(bf)
    )

    # causal masks per slot (identical formula for both cores' block lists)
    def packmask(blocks):
        m = np.zeros((NSLOT, P, KCH), np.float32)
        for j, blk in enumerate(blocks):
            cc = np.arange(KCH)[None, :] + (CJ[j] - 1) * KCH  # key col
            rr = np.arange(P)[:, None] + blk * P              # query row
            m[j] = np.where(cc <= rr, 0.0, MASK_VAL)
        return np.ascontiguousarray(m.transpose(1, 0, 2).astype(bf))  # [P,slot,KCH]

    masks = [packmask(QBLOCKS[0]), packmask(QBLOCKS[1])]

    in_maps = []
    for c in range(N_CORES):
        b, h = divmod(c, 2)
        xb = x[b]  # [S, D]
        xt = np.ascontiguousarray(
            to8(xb).reshape(S, DT, P).transpose(2, 1, 0)
        )
        xnat = np.ascontiguousarray(
            to8(xb).reshape(S // P, P, D).transpose(1, 0, 2)
        )
        xn16 = np.ascontiguousarray(
            xb[: 2 * P].reshape(2, P, D).transpose(1, 0, 2).astype(bf)
        )
        xt16 = np.ascontiguousarray(
            xb[:KCH].reshape(KCH, DT, P).transpose(2, 1, 0).astype(bf)
        )
        rows = np.concatenate(
            [np.arange(blk * P, (blk + 1) * P) for blk in QBLOCKS[h]]
        )
        xq = xb[rows]  # [SQ, D]
        xqt = np.ascontiguousarray(
            to8(xq).reshape(SQ, DT, P).transpose(2, 1, 0)
        )
        xq16 = np.ascontiguousarray(
            xq[:P].reshape(P, DT, P).transpose(2, 1, 0).astype(bf)
        )
        in_maps.append(
            {
                "xT": xt,
                "xn": xnat,
                "xqT": xqt,
                "MT": mp,
                "WvT": wvp,
                "masks": masks[h],
                "MT16": mp16,
                "xq16": xq16,
                "xT16": xt16,
                "xn16": xn16,
            }
        )
    return in_maps


def kernel(x, Wq, Wk, Wv, _spmd_kwargs=None, _results_out=None):
    x = np.asarray(x, dtype=np.float32)
    Wq = np.asarray(Wq, dtype=np.float32)
    Wk = np.asarray(Wk, dtype=np.float32)
    Wv = np.asarray(Wv, dtype=np.float32)
    assert x.shape == (B, S, D)

    nc = _get_nc()
    in_maps = _pack_inputs(x, Wq, Wk, Wv)
    res = run_bass_kernel_spmd(
        nc, in_maps, list(range(N_CORES)), **(_spmd_kwargs or {})
    )
    if _results_out is not None:
        _results_out.append(res)

    out = np.empty((B, S, E), np.float32)
    for c in range(N_CORES):
        b, h = divmod(c, 2)
        o = res.results[c]["out"]
        for j, blk in enumerate(QBLOCKS[h]):
            out[b, blk * P : (blk + 1) * P, :] = o[j * P : (j + 1) * P, :]
    return out


# revision 37
# speedup vs baseline: 1.2375x; 1.0243x over previous
"""Trainium2 Bass kernel for single-head causal attention.

Problem: x[4,2048,1024] f32; Wq/Wk/Wv [1024,1024] (torch Linear layout, y = x@W.T).
  q,k,v = x@W.T ; scores = q@k.T (causal masked, scaled 1/sqrt(1024)) ;
  out = softmax(scores)@v.

Weight folding: scores = xq (Wq^T Wk) xk^T, so with M := 32*(Wq^T Wk)
precomputed on the host the K projection disappears -- x^T itself is the key
matrix. Likewise out = w @ x @ Wv^T, so the V projection collapses to a
per-slot (w.x) @ Wv^T postmultiply.

fp8 everywhere the error averages out: the softmax temperature (1/32) makes
the score path error-tolerant, and PV pass A's quantization noise is iid
across keys so it averages down by sqrt(n_keys). Both run fp8-e4m3 with
DoubleRow perf mode (2 k-tiles per pass, ~1.5x bf16). Pass B (contraction
over d with fresh noise per d: no averaging) stays bf16.

Early rows have few keys -> no averaging, so slot 0 (each core's first query
block = global rows 0..255, 256-key causal extent) runs its whole score +
pass-A path in bf16: a small bf16 q-projection for its 128 queries, bf16
keys, bf16 pass A. This drops max-rel-err from ~1.4e-2 to ~4e-3 (gate 2e-2).

Softmax drops max-subtraction: scores'/1024 = scores/32 is bounded ~1.8 so
exp can't overflow; masked entries underflow to 0; exp runs per-chunk
straight from PSUM with accumulated partial sums.

Sharding: 2 cores per batch, zig-zag query blocks (identical causal extents
[1,8,2,7,3,6,4,5] chunks of 256 on every core -> one SPMD program).

Per-core pipeline (fp32 PSUM accumulation):
  1. qMT = (xq @ M)^T in fp8 DoubleRow; qMT stored fp8.
  2. QK (slots longest-first) fp8 DoubleRow; mask in-place on PSUM edge
     chunk; exp from PSUM on ACT with accumulated sums; weights stored fp8
     (slot 0: bf16 mini-qproj + bf16 QK, weights bf16).
  3. PV pass A fp8 DoubleRow: fp8 PE-transposes of weight block pairs +
     (w @ x8) accumulation, prev slot's (wx) transposes interleaved.
  4. PV pass B bf16: (wx)^T @ Wv^T, 1/sum fused into PSUM->SBUF, DMA out.
"""

from contextlib import ExitStack

import ml_dtypes
import numpy as np

import concourse.mybir as mybir
import concourse.tile as tile
from concourse import bacc
from concourse.bass_utils import run_bass_kernel_spmd
from concourse.masks import make_identity

B, S, D, E = 4, 2048, 1024, 1024
P = 128
N_CORES = 8
DT = D // P          # 8 d-tiles (contraction)
DP = DT // 2         # 4 d-tile PAIRS (fp8 DoubleRow contracts 2 tiles/pass)
SQ = S // 2          # 1024 query rows per core
KCH = 256            # causal-length granularity (key chunk)
NSLOT = SQ // P      # 8 query slots per core

QCH = [512, 512]     # xqT chunking (DoubleRow wants N>=512 passes)
assert sum(QCH) == SQ

# zig-zag query-block assignment: both cores' slots have identical causal
# chunk counts CJ, so one SPMD program serves all cores.
QBLOCKS = [[0, 15, 2, 13, 4, 11, 6, 9], [1, 14, 3, 12, 5, 10, 7, 8]]
CJ = [(b + 1 + 1) // 2 for b in QBLOCKS[0]]  # [1,8,2,7,3,6,4,5]
assert CJ == [(b + 1 + 1) // 2 for b in QBLOCKS[1]]
SLOT_ORDER = sorted(range(NSLOT), key=lambda j: -CJ[j])  # longest first

F32 = mybir.dt.float32
BF16 = mybir.dt.bfloat16
FP8 = mybir.dt.float8e4
DR = mybir.MatmulPerfMode.DoubleRow
AX = mybir.AxisListType.X
EXP = mybir.ActivationFunctionType.Exp
EXP_SCALE = 1.0 / 1024.0   # (1/32 softmax temp) * (1/32 host M-scale)
MASK_VAL = -1.0e9
WPIECES = [(0, 1), (1, 2), (2, 4), (4, 8)]  # M DMA split over out-tiles


def build_kernel():
    nc = bacc.Bacc(
        "TRN2",
        target_bir_lowering=False,
        debug=False,
        num_devices=N_CORES,
        dynamic_dma_scratch_size=64,
    )
    xT_d = nc.dram_tensor("xT", [P, DT, S], FP8, kind="ExternalInput")
    xn_d = nc.dram_tensor("xn", [P, S // P, D], FP8, kind="ExternalInput")
    xqT_d = nc.dram_tensor("xqT", [P, DT, SQ], FP8, kind="ExternalInput")
    m_d = nc.dram_tensor("MT", [P, DT, DT, P], FP8, kind="ExternalInput")
    wv_d = nc.dram_tensor("WvT", [P, DT, E], BF16, kind="ExternalInput")
    msk_d = nc.dram_tensor("masks", [P, NSLOT, KCH], BF16, kind="ExternalInput")
    # bf16 sidecar for slot 0 (rows 0..255): M, first 128 gathered queries,
    # first 256 keys (transposed + natural)
    m16_d = nc.dram_tensor("MT16", [P, DT, DT, P], BF16, kind="ExternalInput")
    xq16_d = nc.dram_tensor("xq16", [P, DT, P], BF16, kind="ExternalInput")
    xT16_d = nc.dram_tensor("xT16", [P, DT, KCH], BF16, kind="ExternalInput")
    xn16_d = nc.dram_tensor("xn16", [P, 2, D], BF16, kind="ExternalInput")
    out_d = nc.dram_tensor("out", [SQ, E], F32, kind="ExternalOutput")

    with tile.TileContext(nc) as tc, ExitStack() as ctx:
        # persistent tensors (right side). Bulk inputs are split into tiles
        # of <=4KB/partition so every dma_start is a single segment with its
        # own semaphore -- larger DMAs get chopped into semaphore-chained
        # segments that occupy the issuing queue until the transfer lands,
        # head-of-line blocking everything behind them (the xbar transposes).
        kqv = ctx.enter_context(tc.tile_pool(name="kqv", bufs=1, side="right"))
        # keys x^T by DoubleRow d-pair: xTt[dp][p, s, k] = x[k, (2dp+s)*128+p]
        xTt = [kqv.tile([P, 2, S], FP8, tag=f"xT{dp}", name=f"xT{dp}") for dp in range(DP)]
        # x natural by key-block group: xnt[a][p, b, d] = x[(4a+b)*128+p, d]
        xnt = [kqv.tile([P, 4, D], FP8, tag=f"xn{a}", name=f"xn{a}") for a in range(4)]
        qMT = kqv.tile([P, DT, SQ], FP8, tag="qMT")      # (xq M)^T (fp8)
        # WvT by d-pair: wvt[h][p, s, e] = Wv[e, (2h+s)*128+p]
        wvt = [kqv.tile([P, 2, E], BF16, tag=f"wv{h}", name=f"wv{h}") for h in range(DP)]
        msk = kqv.tile([P, NSLOT, KCH], BF16, tag="msk")
        # M16 by jt-pair: m16t[g][p, s, d, el] = M[d*128+p, (2g+s)*128+el]
        m16t = [kqv.tile([P, 2, DT, P], BF16, tag=f"m16{g}", name=f"m16{g}") for g in range(DP)]
        xq16 = kqv.tile([P, DT, P], BF16, tag="xq16")
        xT16 = kqv.tile([P, DT, KCH], BF16, tag="xT16")
        xn16 = kqv.tile([P, 2, D], BF16, tag="xn16")
        qMT16 = kqv.tile([P, DT, P], BF16, tag="qMT16")

        # ---------------- folded q projection ----------------
        with (
            tc.tile_pool(name="wpool", bufs=1) as wpool,
            tc.tile_pool(name="xpool", bufs=2) as xpool,
            tc.tile_pool(name="pps", bufs=6, space="PSUM") as pps,
        ):
            # HAM warm-up: dummy matmuls on a zeroed tile fill the DMA-init
            # dead zone and un-throttle the PE clock before real work
            warm = xpool.tile([P, 512], BF16, tag="warm", name="warm", bufs=1)
            nc.gpsimd.memset(warm[:], 0.0)
            wps = pps.tile([P, 512], F32, tag="wps", name="wps", bufs=1)
            for _ in range(10):
                nc.tensor.matmul(
                    wps[:], lhsT=warm[:, 0:P], rhs=warm[:], start=True, stop=True
                )
            for _ in range(6):
                nc.tensor.matmul(
                    wps[:, 0:256],
                    lhsT=warm[:, 0:P],
                    rhs=warm[:, 0:256],
                    start=True,
                    stop=True,
                )

            m_sb = wpool.tile([P, DT, DT, P], FP8, tag="M", name="m_sb")
            lo, hi = WPIECES[0]
            nc.sync.dma_start(m_sb[:, lo:hi], m_d[:, lo:hi])
            xqc = []
            t0 = 0
            for ci, csz in enumerate(QCH):
                xc = xpool.tile([P, DT, 512], FP8, tag="x", name="xc")
                nc.sync.dma_start(xc[:, :, 0:csz], xqT_d[:, :, t0 : t0 + csz])
                xqc.append(xc)
                t0 += csz
                if ci == 0:
                    for lo, hi in WPIECES[1:]:
                        nc.sync.dma_start(m_sb[:, lo:hi], m_d[:, lo:hi])
            # bulk streaming inputs, ordered by first use in the attention
            # phases: xT (QK), masks, slot-0 bf16 sidecar (mini-qproj sits
            # mid-QK), xn (pass A), WvT (pass B) -- all single-segment DMAs
            for dp in range(DP):
                nc.sync.dma_start(xTt[dp][:], xT_d[:, 2 * dp : 2 * dp + 2])
            nc.sync.dma_start(msk[:], msk_d[:])
            for g in range(DP):
                nc.sync.dma_start(m16t[g][:], m16_d[:, 2 * g : 2 * g + 2])
            nc.sync.dma_start(xq16[:], xq16_d[:])
            nc.sync.dma_start(xT16[:], xT16_d[:])
            for a in range(4):
                nc.sync.dma_start(xnt[a][:], xn_d[:, 4 * a : 4 * a + 4])
            nc.sync.dma_start(xn16[:], xn16_d[:])
            for h in range(DP):
                nc.sync.dma_start(wvt[h][:], wv_d[:, 2 * h : 2 * h + 2])

            t0 = 0
            for ci, csz in enumerate(QCH):
                xc = xqc[ci]
                for j_t in range(DT):
                    ps = pps.tile([P, 512], F32, tag="ps", name="ps")
                    for dp in range(DP):
                        nc.tensor.matmul(
                            ps[:, 0:csz],
                            lhsT=m_sb[:, j_t, 2 * dp : 2 * dp + 2, :],
                            rhs=xc[:, 2 * dp : 2 * dp + 2, 0:csz],
                            perf_mode=DR,
                            start=(dp == 0),
                            stop=(dp == DP - 1),
                        )
                    nc.scalar.copy(qMT[:, j_t, t0 : t0 + csz], ps[:, 0:csz])
                t0 += csz

        # ---------------- attention ----------------
        with (
            tc.tile_pool(name="apool", bufs=2) as apool,
            tc.tile_pool(name="wtpool", bufs=4) as wtpool,
            tc.tile_pool(name="wxtpool", bufs=NSLOT) as wxtpool,
            tc.tile_pool(name="stpool", bufs=NSLOT, side="right") as stpool,
            tc.tile_pool(name="c1pool", bufs=1) as c1pool,
        ):
            def emit_scores(j):
                """QK + mask + exp + sum for slot j; transposes the weight
                tile through the DMA xbar (and quantizes to fp8 on DVE for
                the DoubleRow pass-A lhsT). Slot 0 -> bf16 path."""
                C = CJ[j]
                L = C * KCH
                nkb = C * KCH // P
                st = stpool.tile([P, 8], F32, tag="st", name="st")
                if C == 1:
                    # bf16 mini-qproj for slot-0's 128 queries (queued at the
                    # QK tail: SLOT_ORDER puts this slot last)
                    for j_t in range(DT):
                        ps = qkps.tile([P, 512], F32, tag="qk", name="mq")
                        for d in range(DT):
                            nc.tensor.matmul(
                                ps[:, 0:P],
                                lhsT=m16t[j_t // 2][:, j_t % 2, d, :],
                                rhs=xq16[:, d, :],
                                start=(d == 0),
                                stop=(d == DT - 1),
                            )
                        nc.scalar.copy(qMT16[:, j_t, :], ps[:, 0:P])
                    wts = apool.tile(
                        [P, KCH], BF16, tag="wts16", name="wts16", bufs=1
                    )
                    ps = qkps.tile([P, 512], F32, tag="qk", name="qk")
                    for d in range(DT):
                        nc.tensor.matmul(
                            ps[:, 0:KCH],
                            lhsT=qMT16[:, d, :],
                            rhs=xT16[:, d, :],
                            start=(d == 0),
                            stop=(d == DT - 1),
                        )
                    nc.vector.tensor_add(
                        ps[:, 0:KCH], ps[:, 0:KCH], msk[:, j, :]
                    )
                    nc.scalar.activation(
                        wts[:, 0:KCH],
                        ps[:, 0:KCH],
                        EXP,
                        scale=EXP_SCALE,
                        accum_out=st[:, 0:1],
                    )
                    nc.vector.reciprocal(st[:, 7:8], st[:, 0:1])
                    wT = wtpool.tile(
                        [P, 2, P], BF16, tag="wt16s", name="wt16s", bufs=1
                    )
                    nc.sync.dma_start(wT[:], wts[:, 0:KCH], transpose=True)
                    return st, wT

                groups = [(g * 512, 512) for g in range(C // 2)]
                if C % 2:
                    groups.append(((C // 2) * 512, 256))
                wts = apool.tile(
                    [P, S], BF16, tag="wts8", name="wts8", bufs=NSLOT - 1
                )
                nch = len(groups)
                for ci, (k0, ksz) in enumerate(groups):
                    ps = qkps.tile([P, 512], F32, tag="qk", name="qk")
                    for dp in range(DP):
                        nc.tensor.matmul(
                            ps[:, 0:ksz],
                            lhsT=qMT[:, 2 * dp : 2 * dp + 2, j * P : (j + 1) * P],
                            rhs=xTt[dp][:, :, k0 : k0 + ksz],
                            perf_mode=DR,
                            start=(dp == 0),
                            stop=(dp == DP - 1),
                        )
                    if k0 + ksz == L:
                        # causal edge: host mask covers the last 256 keys
                        nc.vector.tensor_add(
                            ps[:, ksz - 256 : ksz],
                            ps[:, ksz - 256 : ksz],
                            msk[:, j, :],
                        )
                    nc.scalar.activation(
                        wts[:, k0 : k0 + ksz],
                        ps[:, 0:ksz],
                        EXP,
                        scale=EXP_SCALE,
                        accum_out=st[:, ci : ci + 1],
                    )
                if nch > 1:
                    nc.vector.tensor_reduce(
                        st[:, 6:7], st[:, 0:nch], axis=AX, op=mybir.AluOpType.add
                    )
                    nc.vector.reciprocal(st[:, 7:8], st[:, 6:7])
                else:
                    nc.vector.reciprocal(st[:, 7:8], st[:, 0:1])
                if j in (1,):
                    # first pass-A slot: their xbar transposes would be
                    # queue-blocked behind the input DMA chains until ~47us;
                    # PE pair-transposes in pass A are cheaper than the stall
                    return st, wts
                wTb = wtpool.tile(
                    [P, nkb, P], BF16, tag=f"wtb{j}", name="wtb", bufs=1
                )
                nc.sync.dma_start(wTb[:], wts[:, 0:L], transpose=True)
                wT = wtpool.tile(
                    [P, nkb, P], FP8, tag=f"wt8{j}", name="wt8", bufs=1
                )
                nc.vector.tensor_copy(wT[:], wTb[:])
                return st, wT

            # emission order tucks slot 0 (whose bf16 mini-qproj + exp is the
            # longest dependency chain) mid-QK so nothing pends at the PSUM
            # pool transition; pass A/B still consume longest-first.
            QK_ORDER = [1, 3, 5, 0, 7, 6, 4, 2]
            with tc.tile_pool(name="qkps", bufs=4, space="PSUM") as qkps:
                scored = {jj: emit_scores(jj) for jj in QK_ORDER}
            staged = [(jj, *scored[jj]) for jj in SLOT_ORDER]

            # ---- PV pass A: w^T @ x8 in fp8 DoubleRow (slot 0 bf16).
            # Slots 1/3 transpose weight-block pairs on the PE here (their
            # xbar DMAs would be queue-blocked); the rest arrive
            # pre-transposed via xbar. (wx) transposes also via xbar; the
            # host packing of WvT matches the d*128+p blocked layout.
            wxT_all = []
            ident = c1pool.tile([P, P], BF16, tag="ident")
            make_identity(nc, ident[:])

            with (
                tc.tile_pool(name="wxps", bufs=6, space="PSUM") as wxps,
                tc.tile_pool(name="trps", bufs=2, space="PSUM") as trps,
            ):
                for si, (j, st, wT) in enumerate(staged):
                    nkb = CJ[j] * KCH // P
                    po = [
                        wxps.tile([P, 512], F32, tag="wx", name=f"wx{ec}")
                        for ec in range(2)
                    ]
                    if CJ[j] == 1:
                        for kb in range(nkb):
                            for ec in range(2):
                                nc.tensor.matmul(
                                    po[ec][:],
                                    lhsT=wT[:, kb, :],
                                    rhs=xn16[:, kb, ec * 512 : (ec + 1) * 512],
                                    start=(kb == 0),
                                    stop=(kb == nkb - 1),
                                )
                    elif j in (1,):
                        # wT here is the raw weight tile: PE pair-transposes
                        npair = nkb // 2
                        wTq = []

                        def emit_trp(kbp, wts=wT):
                            wTp = wtpool.tile(
                                [P, 2, P], FP8, tag="wTp", name="wTp", bufs=4
                            )
                            for i in range(2):
                                kb = 2 * kbp + i
                                pt = trps.tile(
                                    [P, P], BF16, tag="tr", name="pt"
                                )
                                nc.tensor.transpose(
                                    pt[:],
                                    wts[:, kb * P : (kb + 1) * P],
                                    ident[:],
                                )
                                nc.vector.tensor_copy(wTp[:, i, :], pt[:])
                            wTq.append(wTp)

                        emit_trp(0)
                        if npair > 1:
                            emit_trp(1)
                        for kbp in range(npair):
                            if kbp + 2 < npair:
                                emit_trp(kbp + 2)
                            for ec in range(2):
                                nc.tensor.matmul(
                                    po[ec][:],
                                    lhsT=wTq[kbp][:, 0:2, :],
                                    rhs=xnt[kbp // 2][
                                        :,
                                        (2 * kbp) % 4 : (2 * kbp) % 4 + 2,
                                        ec * 512 : (ec + 1) * 512,
                                    ],
                                    perf_mode=DR,
                                    start=(kbp == 0),
                                    stop=(kbp == npair - 1),
                                )
                    else:
                        npair = nkb // 2
                        for kbp in range(npair):
                            for ec in range(2):
                                nc.tensor.matmul(
                                    po[ec][:],
                                    lhsT=wT[:, 2 * kbp : 2 * kbp + 2, :],
                                    rhs=xnt[kbp // 2][
                                        :,
                                        (2 * kbp) % 4 : (2 * kbp) % 4 + 2,
                                        ec * 512 : (ec + 1) * 512,
                                    ],
                                    perf_mode=DR,
                                    start=(kbp == 0),
                                    stop=(kbp == npair - 1),
                                )
                    wx_sb = apool.tile(
                        [P, E], BF16, tag="wx", name="wx_sb", bufs=3
                    )
                    nc.scalar.copy(wx_sb[:, 0:512], po[0][:])
                    nc.vector.tensor_copy(wx_sb[:, 512:1024], po[1][:])
                    wxT = wxtpool.tile([P, DT, P], BF16, tag="wxT", name="wxT")
                    nc.sync.dma_start(wxT[:], wx_sb[:], transpose=True)
                    wxT_all.append(wxT)

            # ---- PV pass B: (wx)^T @ Wv^T, scaled by 1/sum, DMA out.
            with tc.tile_pool(name="pvps", bufs=4, space="PSUM") as pvps:
                for si, (j, st, _) in enumerate(staged):
                    wxT = wxT_all[si]
                    po = [
                        pvps.tile([P, 512], F32, tag="pv", name=f"po{ec}")
                        for ec in range(2)
                    ]
                    for d in range(DT):
                        for ec in range(2):
                            nc.tensor.matmul(
                                po[ec][:],
                                lhsT=wxT[:, d, :],
                                rhs=wvt[d // 2][:, d % 2, ec * 512 : (ec + 1) * 512],
                                start=(d == 0),
                                stop=(d == DT - 1),
                            )
                    ot = apool.tile([P, E], F32, tag="out", name="ot")
                    nc.scalar.mul(ot[:, 0:512], po[0][:], st[:, 7:8])
                    nc.vector.tensor_scalar_mul(
                        ot[:, 512:1024], po[1][:], st[:, 7:8]
                    )
                    nc.sync.dma_start(out_d[j * P : (j + 1) * P, :], ot[:])

    nc.compile()
    return nc


_NC_CACHE = None


def _get_nc():
    global _NC_CACHE
    if _NC_CACHE is None:
        _NC_CACHE = build_kernel()
    return _NC_CACHE


def _pack_inputs(x, Wq, Wk, Wv):
    """Host-side relayout + weight folding."""
    bf = ml_dtypes.bfloat16
    f8 = ml_dtypes.float8_e4m3

    def to8(a):
        return np.clip(a, -240.0, 240.0).astype(f8)

    # folded scores matrix: scores = xq @ M @ xk^T with M = 32*(Wq^T @ Wk)
    # (the 32 re-centers fp8 quantization; the exp scale absorbs it).
    # packed for the q-projection lhsT: mp[p, jt, d, el] = M[d*128+p, jt*128+el]
    M32 = 32.0 * (
        Wq.T.astype(np.float64) @ Wk.astype(np.float64)
    ).astype(np.float32)
    m4 = M32.reshape(DT, P, DT, P).transpose(1, 2, 0, 3)
    mp = np.ascontiguousarray(to8(m4))
    mp16 = np.ascontiguousarray(m4.astype(bf))
    # Wv packed d-outer to match the xbar-transposed (wx)^T layout:
    # wxT[p, d, q] = wx[q, d*128+p], so [p, d, e] = Wv[e, d*128+p]
    wvp = np.ascontiguousarray(
        Wv.reshape(E, DT, P).transpose(2, 1, 0).astype(bf)
    )

    # causal masks per slot (identical formula for both cores' block lists)
    def packmask(blocks):
        m = np.zeros((NSLOT, P, KCH), np.float32)
        for j, blk in enumerate(blocks):
            cc = np.arange(KCH)[None, :] + (CJ[j] - 1) * KCH  # key col
            rr = np.arange(P)[:, None] + blk * P              # query row
            m[j] = np.where(cc <= rr, 0.0, MASK_VAL)
        return np.ascontiguousarray(m.transpose(1, 0, 2).astype(bf))  # [P,slot,KCH]

    masks = [packmask(QBLOCKS[0]), packmask(QBLOCKS[1])]

    in_maps = []
    for c in range(N_CORES):
        b, h = divmod(c, 2)
        xb = x[b]  # [S, D]
        xt = np.ascontiguousarray(
            to8(xb).reshape(S, DT, P).transpose(2, 1, 0)
        )
        xnat = np.ascontiguousarray(
            to8(xb).reshape(S // P, P, D).transpose(1, 0, 2)
        )
        xn16 = np.ascontiguousarray(
            xb[: 2 * P].reshape(2, P, D).transpose(1, 0, 2).astype(bf)
        )
        xt16 = np.ascontiguousarray(
            xb[:KCH].reshape(KCH, DT, P).transpose(2, 1, 0).astype(bf)
        )
        rows = np.concatenate(
            [np.arange(blk * P, (blk + 1) * P) for blk in QBLOCKS[h]]
        )
        xq = xb[rows]  # [SQ, D]
        xqt = np.ascontiguousarray(
            to8(xq).reshape(SQ, DT, P).transpose(2, 1, 0)
        )
        xq16 = np.ascontiguousarray(
            xq[:P].reshape(P, DT, P).transpose(2, 1, 0).astype(bf)
        )
        in_maps.append(
            {
                "xT": xt,
                "xn": xnat,
                "xqT": xqt,
                "MT": mp,
                "WvT": wvp,
                "masks": masks[h],
                "MT16": mp16,
                "xq16": xq16,
                "xT16": xt16,
                "xn16": xn16,
            }
        )
    return in_maps


def kernel(x, Wq, Wk, Wv, _spmd_kwargs=None, _results_out=None):
    x = np.asarray(x, dtype=np.float32)
    Wq = np.asarray(Wq, dtype=np.float32)
    Wk = np.asarray(Wk, dtype=np.float32)
    Wv = np.asarray(Wv, dtype=np.float32)
    assert x.shape == (B, S, D)

    nc = _get_nc()
    in_maps = _pack_inputs(x, Wq, Wk, Wv)
    res = run_bass_kernel_spmd(
        nc, in_maps, list(range(N_CORES)), **(_spmd_kwargs or {})
    )
    if _results_out is not None:
        _results_out.append(res)

    out = np.empty((B, S, E), np.float32)
    for c in range(N_CORES):
        b, h = divmod(c, 2)
        o = res.results[c]["out"]
        for j, blk in enumerate(QBLOCKS[h]):
            out[b, blk * P : (blk + 1) * P, :] = o[j * P : (j + 1) * P, :]
    return out


# revision 39
# speedup vs baseline: 1.2405x; 1.0024x over previous
"""Trainium2 Bass kernel for single-head causal attention.

Problem: x[4,2048,1024] f32; Wq/Wk/Wv [1024,1024] (torch Linear layout, y = x@W.T).
  q,k,v = x@W.T ; scores = q@k.T (causal masked, scaled 1/sqrt(1024)) ;
  out = softmax(scores)@v.

Weight folding: scores = xq (Wq^T Wk) xk^T, so with M := 32*(Wq^T Wk)
precomputed on the host the K projection disappears -- x^T itself is the key
matrix. Likewise out = w @ x @ Wv^T, so the V projection collapses to a
per-slot (w.x) @ Wv^T postmultiply.

fp8 everywhere the error averages out: the softmax temperature (1/32) makes
the score path error-tolerant, and PV pass A's quantization noise is iid
across keys so it averages down by sqrt(n_keys). Both run fp8-e4m3 with
DoubleRow perf mode (2 k-tiles per pass, ~1.5x bf16). Pass B (contraction
over d with fresh noise per d: no averaging) stays bf16.

Early rows have few keys -> no averaging, so slot 0 (each core's first query
block = global rows 0..255, 256-key causal extent) runs its whole score +
pass-A path in bf16: a small bf16 q-projection for its 128 queries, bf16
keys, bf16 pass A. This drops max-rel-err from ~1.4e-2 to ~4e-3 (gate 2e-2).

Softmax drops max-subtraction: scores'/1024 = scores/32 is bounded ~1.8 so
exp can't overflow; masked entries underflow to 0; exp runs per-chunk
straight from PSUM with accumulated partial sums.

Sharding: 2 cores per batch, zig-zag query blocks (identical causal extents
[1,8,2,7,3,6,4,5] chunks of 256 on every core -> one SPMD program).

Per-core pipeline (fp32 PSUM accumulation):
  1. qMT = (xq @ M)^T in fp8 DoubleRow; qMT stored fp8.
  2. QK (slots longest-first) fp8 DoubleRow; mask in-place on PSUM edge
     chunk; exp from PSUM on ACT with accumulated sums; weights stored fp8
     (slot 0: bf16 mini-qproj + bf16 QK, weights bf16).
  3. PV pass A fp8 DoubleRow: fp8 PE-transposes of weight block pairs +
     (w @ x8) accumulation, prev slot's (wx) transposes interleaved.
  4. PV pass B bf16: (wx)^T @ Wv^T, 1/sum fused into PSUM->SBUF, DMA out.
"""

from contextlib import ExitStack

import ml_dtypes
import numpy as np

import concourse.mybir as mybir
import concourse.tile as tile
from concourse import bacc
from concourse.bass_utils import run_bass_kernel_spmd
from concourse.masks import make_identity

B, S, D, E = 4, 2048, 1024, 1024
P = 128
N_CORES = 8
DT = D // P          # 8 d-tiles (contraction)
DP = DT // 2         # 4 d-tile PAIRS (fp8 DoubleRow contracts 2 tiles/pass)
SQ = S // 2          # 1024 query rows per core
KCH = 256            # causal-length granularity (key chunk)
NSLOT = SQ // P      # 8 query slots per core

QCH = [512, 512]     # xqT chunking (DoubleRow wants N>=512 passes)
assert sum(QCH) == SQ

# zig-zag query-block assignment: both cores' slots have identical causal
# chunk counts CJ, so one SPMD program serves all cores.
QBLOCKS = [[0, 15, 2, 13, 4, 11, 6, 9], [1, 14, 3, 12, 5, 10, 7, 8]]
CJ = [(b + 1 + 1) // 2 for b in QBLOCKS[0]]  # [1,8,2,7,3,6,4,5]
assert CJ == [(b + 1 + 1) // 2 for b in QBLOCKS[1]]
SLOT_ORDER = sorted(range(NSLOT), key=lambda j: -CJ[j])  # longest first

F32 = mybir.dt.float32
BF16 = mybir.dt.bfloat16
FP8 = mybir.dt.float8e4
DR = mybir.MatmulPerfMode.DoubleRow
AX = mybir.AxisListType.X
EXP = mybir.ActivationFunctionType.Exp
EXP_SCALE = 1.0 / 1024.0   # (1/32 softmax temp) * (1/32 host M-scale)
MASK_VAL = -1.0e9
WPIECES = [(0, 1), (1, 2), (2, 4), (4, 8)]  # M DMA split over out-tiles


def build_kernel():
    nc = bacc.Bacc(
        "TRN2",
        target_bir_lowering=False,
        debug=False,
        num_devices=N_CORES,
        dynamic_dma_scratch_size=64,
    )
    xT_d = nc.dram_tensor("xT", [P, DT, S], FP8, kind="ExternalInput")
    xn_d = nc.dram_tensor("xn", [P, S // P, D], FP8, kind="ExternalInput")
    xqT_d = nc.dram_tensor("xqT", [P, DT, SQ], FP8, kind="ExternalInput")
    m_d = nc.dram_tensor("MT", [P, DT, DT, P], FP8, kind="ExternalInput")
    wv_d = nc.dram_tensor("WvT", [P, DT, E], BF16, kind="ExternalInput")
    msk_d = nc.dram_tensor("masks", [P, NSLOT, KCH], BF16, kind="ExternalInput")
    # bf16 sidecar for slot 0 (rows 0..255): M, first 128 gathered queries,
    # first 256 keys (transposed + natural)
    m16_d = nc.dram_tensor("MT16", [P, DT, DT, P], BF16, kind="ExternalInput")
    xq16_d = nc.dram_tensor("xq16", [P, DT, P], BF16, kind="ExternalInput")
    xT16_d = nc.dram_tensor("xT16", [P, DT, KCH], BF16, kind="ExternalInput")
    xn16_d = nc.dram_tensor("xn16", [P, 2, D], BF16, kind="ExternalInput")
    out_d = nc.dram_tensor("out", [SQ, E], F32, kind="ExternalOutput")

    with tile.TileContext(nc) as tc, ExitStack() as ctx:
        # persistent tensors (right side). Bulk inputs are split into tiles
        # of <=4KB/partition so every dma_start is a single segment with its
        # own semaphore -- larger DMAs get chopped into semaphore-chained
        # segments that occupy the issuing queue until the transfer lands,
        # head-of-line blocking everything behind them (the xbar transposes).
        kqv = ctx.enter_context(tc.tile_pool(name="kqv", bufs=1, side="right"))
        # keys x^T by DoubleRow d-pair: xTt[dp][p, s, k] = x[k, (2dp+s)*128+p]
        xTt = [kqv.tile([P, 2, S], FP8, tag=f"xT{dp}", name=f"xT{dp}") for dp in range(DP)]
        # x natural by key-block group: xnt[a][p, b, d] = x[(4a+b)*128+p, d]
        xnt = [kqv.tile([P, 4, D], FP8, tag=f"xn{a}", name=f"xn{a}") for a in range(4)]
        qMT = kqv.tile([P, DT, SQ], FP8, tag="qMT")      # (xq M)^T (fp8)
        # WvT by d-pair: wvt[h][p, s, e] = Wv[e, (2h+s)*128+p]
        wvt = [kqv.tile([P, 2, E], BF16, tag=f"wv{h}", name=f"wv{h}") for h in range(DP)]
        msk = kqv.tile([P, NSLOT, KCH], BF16, tag="msk")
        # M16 by jt-pair: m16t[g][p, s, d, el] = M[d*128+p, (2g+s)*128+el]
        m16t = [kqv.tile([P, 2, DT, P], BF16, tag=f"m16{g}", name=f"m16{g}") for g in range(DP)]
        xq16 = kqv.tile([P, DT, P], BF16, tag="xq16")
        xT16 = kqv.tile([P, DT, KCH], BF16, tag="xT16")
        xn16 = kqv.tile([P, 2, D], BF16, tag="xn16")
        qMT16 = kqv.tile([P, DT, P], BF16, tag="qMT16")

        # ---------------- folded q projection ----------------
        with (
            tc.tile_pool(name="wpool", bufs=1) as wpool,
            tc.tile_pool(name="xpool", bufs=2) as xpool,
            tc.tile_pool(name="pps", bufs=6, space="PSUM") as pps,
        ):
            # HAM warm-up: dummy matmuls on a zeroed tile fill the DMA-init
            # dead zone and un-throttle the PE clock before real work
            warm = xpool.tile([P, 512], BF16, tag="warm", name="warm", bufs=1)
            nc.gpsimd.memset(warm[:], 0.0)
            wps = pps.tile([P, 512], F32, tag="wps", name="wps", bufs=1)
            for _ in range(10):
                nc.tensor.matmul(
                    wps[:], lhsT=warm[:, 0:P], rhs=warm[:], start=True, stop=True
                )
            for _ in range(6):
                nc.tensor.matmul(
                    wps[:, 0:256],
                    lhsT=warm[:, 0:P],
                    rhs=warm[:, 0:256],
                    start=True,
                    stop=True,
                )

            m_sb = wpool.tile([P, DT, DT, P], FP8, tag="M", name="m_sb")
            lo, hi = WPIECES[0]
            nc.sync.dma_start(m_sb[:, lo:hi], m_d[:, lo:hi])
            xqc = []
            t0 = 0
            for ci, csz in enumerate(QCH):
                xc = xpool.tile([P, DT, 512], FP8, tag="x", name="xc")
                nc.sync.dma_start(xc[:, :, 0:csz], xqT_d[:, :, t0 : t0 + csz])
                xqc.append(xc)
                t0 += csz
                if ci == 0:
                    for lo, hi in WPIECES[1:]:
                        nc.sync.dma_start(m_sb[:, lo:hi], m_d[:, lo:hi])
            # bulk streaming inputs, ordered by first use in the attention
            # phases: xT (QK), masks, slot-0 bf16 sidecar (mini-qproj sits
            # mid-QK), xn (pass A), WvT (pass B) -- all single-segment DMAs
            for dp in range(DP):
                nc.sync.dma_start(xTt[dp][:], xT_d[:, 2 * dp : 2 * dp + 2])
            nc.sync.dma_start(msk[:], msk_d[:])
            for g in range(DP):
                nc.sync.dma_start(m16t[g][:], m16_d[:, 2 * g : 2 * g + 2])
            nc.sync.dma_start(xq16[:], xq16_d[:])
            nc.sync.dma_start(xT16[:], xT16_d[:])
            for a in range(4):
                nc.sync.dma_start(xnt[a][:], xn_d[:, 4 * a : 4 * a + 4])
            nc.sync.dma_start(xn16[:], xn16_d[:])
            for h in range(DP):
                nc.sync.dma_start(wvt[h][:], wv_d[:, 2 * h : 2 * h + 2])

            t0 = 0
            for ci, csz in enumerate(QCH):
                xc = xqc[ci]
                for j_t in range(DT):
                    ps = pps.tile([P, 512], F32, tag="ps", name="ps")
                    for dp in range(DP):
                        nc.tensor.matmul(
                            ps[:, 0:csz],
                            lhsT=m_sb[:, j_t, 2 * dp : 2 * dp + 2, :],
                            rhs=xc[:, 2 * dp : 2 * dp + 2, 0:csz],
                            perf_mode=DR,
                            start=(dp == 0),
                            stop=(dp == DP - 1),
                        )
                    nc.scalar.copy(qMT[:, j_t, t0 : t0 + csz], ps[:, 0:csz])
                t0 += csz

        # ---------------- attention ----------------
        with (
            tc.tile_pool(name="apool", bufs=2) as apool,
            tc.tile_pool(name="wtpool", bufs=4) as wtpool,
            tc.tile_pool(name="wxtpool", bufs=NSLOT) as wxtpool,
            tc.tile_pool(name="stpool", bufs=NSLOT, side="right") as stpool,
            tc.tile_pool(name="c1pool", bufs=1) as c1pool,
        ):
            def emit_scores(j):
                """QK + mask + exp + sum for slot j; transposes the weight
                tile through the DMA xbar (and quantizes to fp8 on DVE for
                the DoubleRow pass-A lhsT). Slot 0 -> bf16 path."""
                C = CJ[j]
                L = C * KCH
                nkb = C * KCH // P
                st = stpool.tile([P, 8], F32, tag="st", name="st")
                if C == 1:
                    # bf16 mini-qproj for slot-0's 128 queries (queued at the
                    # QK tail: SLOT_ORDER puts this slot last)
                    for j_t in range(DT):
                        ps = qkps.tile([P, 512], F32, tag="qk", name="mq")
                        for d in range(DT):
                            nc.tensor.matmul(
                                ps[:, 0:P],
                                lhsT=m16t[j_t // 2][:, j_t % 2, d, :],
                                rhs=xq16[:, d, :],
                                start=(d == 0),
                                stop=(d == DT - 1),
                            )
                        nc.scalar.copy(qMT16[:, j_t, :], ps[:, 0:P])
                    wts = apool.tile(
                        [P, KCH], BF16, tag="wts16", name="wts16", bufs=1
                    )
                    ps = qkps.tile([P, 512], F32, tag="qk", name="qk")
                    for d in range(DT):
                        nc.tensor.matmul(
                            ps[:, 0:KCH],
                            lhsT=qMT16[:, d, :],
                            rhs=xT16[:, d, :],
                            start=(d == 0),
                            stop=(d == DT - 1),
                        )
                    nc.vector.tensor_add(
                        ps[:, 0:KCH], ps[:, 0:KCH], msk[:, j, :]
                    )
                    nc.scalar.activation(
                        wts[:, 0:KCH],
                        ps[:, 0:KCH],
                        EXP,
                        scale=EXP_SCALE,
                        accum_out=st[:, 0:1],
                    )
                    nc.vector.reciprocal(st[:, 7:8], st[:, 0:1])
                    wT = wtpool.tile(
                        [P, 2, P], BF16, tag="wt16s", name="wt16s", bufs=1
                    )
                    nc.sync.dma_start(wT[:], wts[:, 0:KCH], transpose=True)
                    return st, wT

                groups = [(g * 512, 512) for g in range(C // 2)]
                if C % 2:
                    groups.append(((C // 2) * 512, 256))
                wts = apool.tile(
                    [P, S], BF16, tag="wts8", name="wts8", bufs=NSLOT - 1
                )
                nch = len(groups)
                for ci, (k0, ksz) in enumerate(groups):
                    ps = qkps.tile([P, 512], F32, tag="qk", name="qk")
                    for dp in range(DP):
                        nc.tensor.matmul(
                            ps[:, 0:ksz],
                            lhsT=qMT[:, 2 * dp : 2 * dp + 2, j * P : (j + 1) * P],
                            rhs=xTt[dp][:, :, k0 : k0 + ksz],
                            perf_mode=DR,
                            start=(dp == 0),
                            stop=(dp == DP - 1),
                        )
                    if k0 + ksz == L:
                        # causal edge: host mask covers the last 256 keys
                        nc.vector.tensor_add(
                            ps[:, ksz - 256 : ksz],
                            ps[:, ksz - 256 : ksz],
                            msk[:, j, :],
                        )
                    nc.scalar.activation(
                        wts[:, k0 : k0 + ksz],
                        ps[:, 0:ksz],
                        EXP,
                        scale=EXP_SCALE,
                        accum_out=st[:, ci : ci + 1],
                    )
                if nch > 1:
                    nc.vector.tensor_reduce(
                        st[:, 6:7], st[:, 0:nch], axis=AX, op=mybir.AluOpType.add
                    )
                    nc.vector.reciprocal(st[:, 7:8], st[:, 6:7])
                else:
                    nc.vector.reciprocal(st[:, 7:8], st[:, 0:1])
                if j in (1,):
                    # first pass-A slot: their xbar transposes would be
                    # queue-blocked behind the input DMA chains until ~47us;
                    # PE pair-transposes in pass A are cheaper than the stall
                    return st, wts
                wTb = wtpool.tile(
                    [P, nkb, P], BF16, tag=f"wtb{j}", name="wtb", bufs=1
                )
                nc.sync.dma_start(wTb[:], wts[:, 0:L], transpose=True)
                wT = wtpool.tile(
                    [P, nkb, P], FP8, tag=f"wt8{j}", name="wt8", bufs=1
                )
                nc.vector.tensor_copy(wT[:], wTb[:])
                return st, wT

            # emission order tucks slot 0 (whose bf16 mini-qproj + exp is the
            # longest dependency chain) mid-QK so nothing pends at the PSUM
            # pool transition; pass A/B still consume longest-first.
            QK_ORDER = [1, 3, 5, 0, 7, 6, 4, 2]
            with tc.tile_pool(name="qkps", bufs=4, space="PSUM") as qkps:
                scored = {jj: emit_scores(jj) for jj in QK_ORDER}
            staged = [(jj, *scored[jj]) for jj in SLOT_ORDER]

            # ---- PV pass A: w^T @ x8 in fp8 DoubleRow (slot 0 bf16).
            # Slots 1/3 transpose weight-block pairs on the PE here (their
            # xbar DMAs would be queue-blocked); the rest arrive
            # pre-transposed via xbar. (wx) transposes also via xbar; the
            # host packing of WvT matches the d*128+p blocked layout.
            wxT_all = []
            ident = c1pool.tile([P, P], BF16, tag="ident")
            make_identity(nc, ident[:])

            with (
                tc.tile_pool(name="wxps", bufs=6, space="PSUM") as wxps,
                tc.tile_pool(name="trps", bufs=2, space="PSUM") as trps,
            ):
                for si, (j, st, wT) in enumerate(staged):
                    nkb = CJ[j] * KCH // P
                    po = [
                        wxps.tile([P, 512], F32, tag="wx", name=f"wx{ec}")
                        for ec in range(2)
                    ]
                    if CJ[j] == 1:
                        for kb in range(nkb):
                            for ec in range(2):
                                nc.tensor.matmul(
                                    po[ec][:],
                                    lhsT=wT[:, kb, :],
                                    rhs=xn16[:, kb, ec * 512 : (ec + 1) * 512],
                                    start=(kb == 0),
                                    stop=(kb == nkb - 1),
                                )
                    elif j in (1,):
                        # wT here is the raw weight tile: PE pair-transposes
                        npair = nkb // 2
                        wTq = []

                        def emit_trp(kbp, wts=wT):
                            wTp = wtpool.tile(
                                [P, 2, P], FP8, tag="wTp", name="wTp", bufs=4
                            )
                            for i in range(2):
                                kb = 2 * kbp + i
                                pt = trps.tile(
                                    [P, P], BF16, tag="tr", name="pt"
                                )
                                nc.tensor.transpose(
                                    pt[:],
                                    wts[:, kb * P : (kb + 1) * P],
                                    ident[:],
                                )
                                nc.vector.tensor_copy(wTp[:, i, :], pt[:])
                            wTq.append(wTp)

                        emit_trp(0)
                        if npair > 1:
                            emit_trp(1)
                        for kbp in range(npair):
                            if kbp + 2 < npair:
                                emit_trp(kbp + 2)
                            for ec in range(2):
                                nc.tensor.matmul(
                                    po[ec][:],
                                    lhsT=wTq[kbp][:, 0:2, :],
                                    rhs=xnt[kbp // 2][
                                        :,
                                        (2 * kbp) % 4 : (2 * kbp) % 4 + 2,
                                        ec * 512 : (ec + 1) * 512,
                                    ],
                                    perf_mode=DR,
                                    start=(kbp == 0),
                                    stop=(kbp == npair - 1),
                                )
                    else:
                        npair = nkb // 2
                        for kbp in range(npair):
                            for ec in range(2):
                                nc.tensor.matmul(
                                    po[ec][:],
                                    lhsT=wT[:, 2 * kbp : 2 * kbp + 2, :],
                                    rhs=xnt[kbp // 2][
                                        :,
                                        (2 * kbp) % 4 : (2 * kbp) % 4 + 2,
                                        ec * 512 : (ec + 1) * 512,
                                    ],
                                    perf_mode=DR,
                                    start=(kbp == 0),
                                    stop=(kbp == npair - 1),
                                )
                    wx_sb = apool.tile(
                        [P, E], BF16, tag="wx", name="wx_sb", bufs=3
                    )
                    nc.scalar.copy(wx_sb[:, 0:512], po[0][:])
                    nc.vector.tensor_copy(wx_sb[:, 512:1024], po[1][:])
                    wxT = wxtpool.tile([P, DT, P], BF16, tag="wxT", name="wxT")
                    nc.sync.dma_start(wxT[:], wx_sb[:], transpose=True)
                    wxT_all.append(wxT)

            # ---- PV pass B: (wx)^T @ Wv^T, scaled by 1/sum, DMA out.
            with tc.tile_pool(name="pvps", bufs=4, space="PSUM") as pvps:
                for si, (j, st, _) in enumerate(staged):
                    wxT = wxT_all[si]
                    po = [
                        pvps.tile([P, 512], F32, tag="pv", name=f"po{ec}")
                        for ec in range(2)
                    ]
                    for d in range(DT):
                        for ec in range(2):
                            nc.tensor.matmul(
                                po[ec][:],
                                lhsT=wxT[:, d, :],
                                rhs=wvt[d // 2][:, d % 2, ec * 512 : (ec + 1) * 512],
                                start=(d == 0),
                                stop=(d == DT - 1),
                            )
                    ot = apool.tile([P, E], F32, tag="out", name="ot")
                    nc.scalar.mul(ot[:, 0:512], po[0][:], st[:, 7:8])
                    nc.vector.tensor_scalar_mul(
                        ot[:, 512:1024], po[1][:], st[:, 7:8]
                    )
                    nc.sync.dma_start(out_d[j * P : (j + 1) * P, :], ot[:])

    nc.compile()
    return nc


_NC_CACHE = None


def _get_nc():
    global _NC_CACHE
    if _NC_CACHE is None:
        _NC_CACHE = build_kernel()
    return _NC_CACHE


def _pack_inputs(x, Wq, Wk, Wv):
    """Host-side relayout + weight folding."""
    bf = ml_dtypes.bfloat16
    f8 = ml_dtypes.float8_e4m3

    def to8(a):
        return np.clip(a, -240.0, 240.0).astype(f8)

    # folded scores matrix: scores = xq @ M @ xk^T with M = 32*(Wq^T @ Wk)
    # (the 32 re-centers fp8 quantization; the exp scale absorbs it).
    # packed for the q-projection lhsT: mp[p, jt, d, el] = M[d*128+p, jt*128+el]
    M32 = 32.0 * (
        Wq.T.astype(np.float64) @ Wk.astype(np.float64)
    ).astype(np.float32)
    m4 = M32.reshape(DT, P, DT, P).transpose(1, 2, 0, 3)
    mp = np.ascontiguousarray(to8(m4))
    mp16 = np.ascontiguousarray(m4.astype(bf))
    # Wv packed d-outer to match the xbar-transposed (wx)^T layout:
    # wxT[p, d, q] = wx[q, d*128+p], so [p, d, e] = Wv[e, d*128+p]
    wvp = np.ascontiguousarray(
        Wv.reshape(E, DT, P).transpose(2, 1, 0).astype(bf)
    )

    # causal masks per slot (identical formula for both cores' block lists)
    def packmask(blocks):
        m = np.zeros((NSLOT, P, KCH), np.float32)
        for j, blk in enumerate(blocks):
            cc = np.arange(KCH)[None, :] + (CJ[j] - 1) * KCH  # key col
            rr = np.arange(P)[:, None] + blk * P              # query row
            m[j] = np.where(cc <= rr, 0.0, MASK_VAL)
        return np.ascontiguousarray(m.transpose(1, 0, 2).astype(bf))  # [P,slot,KCH]

    masks = [packmask(QBLOCKS[0]), packmask(QBLOCKS[1])]

    in_maps = []
    for c in range(N_CORES):
        b, h = divmod(c, 2)
        xb = x[b]  # [S, D]
        xt = np.ascontiguousarray(
            to8(xb).reshape(S, DT, P).transpose(2, 1, 0)
        )
        xnat = np.ascontiguousarray(
            to8(xb).reshape(S // P, P, D).transpose(1, 0, 2)
        )
        xn16 = np.ascontiguousarray(
            xb[: 2 * P].reshape(2, P, D).transpose(1, 0, 2).astype(bf)
        )
        xt16 = np.ascontiguousarray(
            xb[:KCH].reshape(KCH, DT, P).transpose(2, 1, 0).astype(bf)
        )
        rows = np.concatenate(
            [np.arange(blk * P, (blk + 1) * P) for blk in QBLOCKS[h]]
        )
        xq = xb[rows]  # [SQ, D]
        xqt = np.ascontiguousarray(
            to8(xq).reshape(SQ, DT, P).transpose(2, 1, 0)
        )
        xq16 = np.ascontiguousarray(
            xq[:P].reshape(P, DT, P).transpose(2, 1, 0).astype(bf)
        )
        in_maps.append(
            {
                "xT": xt,
                "xn": xnat,
                "xqT": xqt,
                "MT": mp,
                "WvT": wvp,
                "masks": masks[h],
                "MT16": mp16,
                "xq16": xq16,
                "xT16": xt16,
                "xn16": xn16,
            }
        )
    return in_maps


def kernel(x, Wq, Wk, Wv, _spmd_kwargs=None, _results_out=None):
    x = np.asarray(x, dtype=np.float32)
    Wq = np.asarray(Wq, dtype=np.float32)
    Wk = np.asarray(Wk, dtype=np.float32)
    Wv = np.asarray(Wv, dtype=np.float32)
    assert x.shape == (B, S, D)

    nc = _get_nc()
    in_maps = _pack_inputs(x, Wq, Wk, Wv)
    res = run_bass_kernel_spmd(
        nc, in_maps, list(range(N_CORES)), **(_spmd_kwargs or {})
    )
    if _results_out is not None:
        _results_out.append(res)

    out = np.empty((B, S, E), np.float32)
    for c in range(N_CORES):
        b, h = divmod(c, 2)
        o = res.results[c]["out"]
        for j, blk in enumerate(QBLOCKS[h]):
            out[b, blk * P : (blk + 1) * P, :] = o[j * P : (j + 1) * P, :]
    return out


# revision 40
# speedup vs baseline: 1.2961x; 1.0448x over previous
"""Trainium2 Bass kernel for single-head causal attention.

Problem: x[4,2048,1024] f32; Wq/Wk/Wv [1024,1024] (torch Linear layout, y = x@W.T).
  q,k,v = x@W.T ; scores = q@k.T (causal masked, scaled 1/sqrt(1024)) ;
  out = softmax(scores)@v.

Weight folding: scores = xq (Wq^T Wk) xk^T, so with M := 32*(Wq^T Wk)
precomputed on the host the K projection disappears -- x^T itself is the key
matrix. Likewise out = w @ x @ Wv^T, so the V projection collapses to a
per-slot (w.x) @ Wv^T postmultiply.

fp8 everywhere the error averages out: the softmax temperature (1/32) makes
the score path error-tolerant, and PV pass A's quantization noise is iid
across keys so it averages down by sqrt(n_keys). Both run fp8-e4m3 with
DoubleRow perf mode (2 k-tiles per pass, ~1.5x bf16). Pass B (contraction
over d with fresh noise per d: no averaging) stays bf16.

Early rows have few keys -> no averaging, so slot 0 (each core's first query
block = global rows 0..255, 256-key causal extent) runs its whole score +
pass-A path in bf16: a small bf16 q-projection for its 128 queries, bf16
keys, bf16 pass A. This drops max-rel-err from ~1.4e-2 to ~4e-3 (gate 2e-2).

Softmax drops max-subtraction: scores'/1024 = scores/32 is bounded ~1.8 so
exp can't overflow; masked entries underflow to 0; exp runs per-chunk
straight from PSUM with accumulated partial sums.

Sharding: 2 cores per batch, zig-zag query blocks (identical causal extents
[1,8,2,7,3,6,4,5] chunks of 256 on every core -> one SPMD program).

Per-core pipeline (fp32 PSUM accumulation):
  1. qMT = (xq @ M)^T in fp8 DoubleRow; qMT stored fp8.
  2. QK (slots longest-first) fp8 DoubleRow; mask in-place on PSUM edge
     chunk; exp from PSUM on ACT with accumulated sums; weights stored fp8
     (slot 0: bf16 mini-qproj + bf16 QK, weights bf16).
  3. PV pass A fp8 DoubleRow: fp8 PE-transposes of weight block pairs +
     (w @ x8) accumulation, prev slot's (wx) transposes interleaved.
  4. PV pass B bf16: (wx)^T @ Wv^T, 1/sum fused into PSUM->SBUF, DMA out.
"""

from contextlib import ExitStack

import ml_dtypes
import numpy as np

import concourse.mybir as mybir
import concourse.tile as tile
from concourse import bacc
from concourse.bass_utils import run_bass_kernel_spmd
from concourse.masks import make_identity

B, S, D, E = 4, 2048, 1024, 1024
P = 128
N_CORES = 8
DT = D // P          # 8 d-tiles (contraction)
DP = DT // 2         # 4 d-tile PAIRS (fp8 DoubleRow contracts 2 tiles/pass)
SQ = S // 2          # 1024 query rows per core
KCH = 256            # causal-length granularity (key chunk)
NSLOT = SQ // P      # 8 query slots per core

QCH = [512, 512]     # xqT chunking (DoubleRow wants N>=512 passes)
assert sum(QCH) == SQ

# zig-zag query-block assignment: both cores' slots have identical causal
# chunk counts CJ, so one SPMD program serves all cores.
QBLOCKS = [[0, 15, 2, 13, 4, 11, 6, 9], [1, 14, 3, 12, 5, 10, 7, 8]]
CJ = [(b + 1 + 1) // 2 for b in QBLOCKS[0]]  # [1,8,2,7,3,6,4,5]
assert CJ == [(b + 1 + 1) // 2 for b in QBLOCKS[1]]
SLOT_ORDER = sorted(range(NSLOT), key=lambda j: -CJ[j])  # longest first

F32 = mybir.dt.float32
BF16 = mybir.dt.bfloat16
FP8 = mybir.dt.float8e4
DR = mybir.MatmulPerfMode.DoubleRow
AX = mybir.AxisListType.X
EXP = mybir.ActivationFunctionType.Exp
EXP_SCALE = 1.0 / 1024.0   # (1/32 softmax temp) * (1/32 host M-scale)
MASK_VAL = -1.0e9
WPIECES = [(0, 1), (1, 2), (2, 4), (4, 8)]  # M DMA split over out-tiles


def build_kernel():
    nc = bacc.Bacc(
        "TRN2",
        target_bir_lowering=False,
        debug=False,
        num_devices=N_CORES,
        dynamic_dma_scratch_size=64,
    )
    xT_d = nc.dram_tensor("xT", [P, DT, S], FP8, kind="ExternalInput")
    xn_d = nc.dram_tensor("xn", [P, S // P, D], FP8, kind="ExternalInput")
    xqT_d = nc.dram_tensor("xqT", [P, DT, SQ], FP8, kind="ExternalInput")
    m_d = nc.dram_tensor("MT", [P, DT, DT, P], FP8, kind="ExternalInput")
    wv_d = nc.dram_tensor("WvT", [P, DT, E], BF16, kind="ExternalInput")
    msk_d = nc.dram_tensor("masks", [P, NSLOT, KCH], BF16, kind="ExternalInput")
    # bf16 sidecar for slot 0 (rows 0..255): M, first 128 gathered queries,
    # first 256 keys (transposed + natural)
    m16_d = nc.dram_tensor("MT16", [P, DT, DT, P], BF16, kind="ExternalInput")
    xq16_d = nc.dram_tensor("xq16", [P, DT, P], BF16, kind="ExternalInput")
    xT16_d = nc.dram_tensor("xT16", [P, DT, KCH], BF16, kind="ExternalInput")
    xn16_d = nc.dram_tensor("xn16", [P, 2, D], BF16, kind="ExternalInput")
    out_d = nc.dram_tensor("out", [SQ, E], F32, kind="ExternalOutput")

    with tile.TileContext(nc) as tc, ExitStack() as ctx:
        # persistent tensors (right side). Bulk inputs are split into tiles
        # of <=4KB/partition so every dma_start is a single segment with its
        # own semaphore -- larger DMAs get chopped into semaphore-chained
        # segments that occupy the issuing queue until the transfer lands,
        # head-of-line blocking everything behind them (the xbar transposes).
        kqv = ctx.enter_context(tc.tile_pool(name="kqv", bufs=1, side="right"))
        # keys x^T by DoubleRow d-pair: xTt[dp][p, s, k] = x[k, (2dp+s)*128+p]
        xTt = [kqv.tile([P, 2, S], FP8, tag=f"xT{dp}", name=f"xT{dp}") for dp in range(DP)]
        # x natural by key-block group: xnt[a][p, b, d] = x[(4a+b)*128+p, d]
        xnt = [kqv.tile([P, 4, D], FP8, tag=f"xn{a}", name=f"xn{a}") for a in range(4)]
        qMT = kqv.tile([P, DT, SQ], FP8, tag="qMT")      # (xq M)^T (fp8)
        # WvT by d-pair: wvt[h][p, s, e] = Wv[e, (2h+s)*128+p]
        wvt = [kqv.tile([P, 2, E], BF16, tag=f"wv{h}", name=f"wv{h}") for h in range(DP)]
        msk = kqv.tile([P, NSLOT, KCH], BF16, tag="msk")
        # M16 by jt-pair: m16t[g][p, s, d, el] = M[d*128+p, (2g+s)*128+el]
        m16t = [kqv.tile([P, 2, DT, P], BF16, tag=f"m16{g}", name=f"m16{g}") for g in range(DP)]
        xq16 = kqv.tile([P, DT, P], BF16, tag="xq16")
        xT16 = kqv.tile([P, DT, KCH], BF16, tag="xT16")
        xn16 = kqv.tile([P, 2, D], BF16, tag="xn16")
        qMT16 = kqv.tile([P, DT, P], BF16, tag="qMT16")

        # ---------------- folded q projection ----------------
        with (
            tc.tile_pool(name="wpool", bufs=1) as wpool,
            tc.tile_pool(name="xpool", bufs=2) as xpool,
            tc.tile_pool(name="pps", bufs=6, space="PSUM") as pps,
        ):
            # HAM warm-up: dummy matmuls on a zeroed tile fill the DMA-init
            # dead zone and un-throttle the PE clock before real work
            warm = xpool.tile([P, 512], BF16, tag="warm", name="warm", bufs=1)
            nc.gpsimd.memset(warm[:], 0.0)
            wps = pps.tile([P, 512], F32, tag="wps", name="wps", bufs=1)
            for _ in range(8):
                nc.tensor.matmul(
                    wps[:], lhsT=warm[:, 0:P], rhs=warm[:], start=True, stop=True
                )
            for _ in range(6):
                nc.tensor.matmul(
                    wps[:, 0:256],
                    lhsT=warm[:, 0:P],
                    rhs=warm[:, 0:256],
                    start=True,
                    stop=True,
                )

            m_sb = wpool.tile([P, DT, DT, P], FP8, tag="M", name="m_sb")
            lo, hi = WPIECES[0]
            nc.sync.dma_start(m_sb[:, lo:hi], m_d[:, lo:hi])
            xqc = []
            t0 = 0
            for ci, csz in enumerate(QCH):
                xc = xpool.tile([P, DT, 512], FP8, tag="x", name="xc")
                nc.sync.dma_start(xc[:, :, 0:csz], xqT_d[:, :, t0 : t0 + csz])
                xqc.append(xc)
                t0 += csz
                if ci == 0:
                    for lo, hi in WPIECES[1:]:
                        nc.sync.dma_start(m_sb[:, lo:hi], m_d[:, lo:hi])
            # bulk streaming inputs, ordered by first use in the attention
            # phases: xT (QK), masks, slot-0 bf16 sidecar (mini-qproj sits
            # mid-QK), xn (pass A), WvT (pass B) -- all single-segment DMAs
            for dp in range(DP):
                nc.sync.dma_start(xTt[dp][:], xT_d[:, 2 * dp : 2 * dp + 2])
            nc.sync.dma_start(msk[:], msk_d[:])
            for g in range(DP):
                nc.sync.dma_start(m16t[g][:], m16_d[:, 2 * g : 2 * g + 2])
            nc.sync.dma_start(xq16[:], xq16_d[:])
            nc.sync.dma_start(xT16[:], xT16_d[:])
            for a in range(4):
                nc.sync.dma_start(xnt[a][:], xn_d[:, 4 * a : 4 * a + 4])
            nc.sync.dma_start(xn16[:], xn16_d[:])
            for h in range(DP):
                nc.sync.dma_start(wvt[h][:], wv_d[:, 2 * h : 2 * h + 2])

            t0 = 0
            for ci, csz in enumerate(QCH):
                xc = xqc[ci]
                for j_t in range(DT):
                    ps = pps.tile([P, 512], F32, tag="ps", name="ps")
                    for dp in range(DP):
                        nc.tensor.matmul(
                            ps[:, 0:csz],
                            lhsT=m_sb[:, j_t, 2 * dp : 2 * dp + 2, :],
                            rhs=xc[:, 2 * dp : 2 * dp + 2, 0:csz],
                            perf_mode=DR,
                            start=(dp == 0),
                            stop=(dp == DP - 1),
                        )
                    nc.scalar.copy(qMT[:, j_t, t0 : t0 + csz], ps[:, 0:csz])
                t0 += csz

        # ---------------- attention ----------------
        with (
            tc.tile_pool(name="apool", bufs=2) as apool,
            tc.tile_pool(name="wtpool", bufs=4) as wtpool,
            tc.tile_pool(name="wxtpool", bufs=NSLOT) as wxtpool,
            tc.tile_pool(name="stpool", bufs=NSLOT, side="right") as stpool,
            tc.tile_pool(name="c1pool", bufs=1) as c1pool,
        ):
            def emit_scores(j):
                """QK + mask + exp + sum for slot j; transposes the weight
                tile through the DMA xbar (and quantizes to fp8 on DVE for
                the DoubleRow pass-A lhsT). Slot 0 -> bf16 path."""
                C = CJ[j]
                L = C * KCH
                nkb = C * KCH // P
                st = stpool.tile([P, 8], F32, tag="st", name="st")
                if C == 1:
                    # bf16 mini-qproj for slot-0's 128 queries (queued at the
                    # QK tail: SLOT_ORDER puts this slot last)
                    for j_t in range(DT):
                        ps = qkps.tile([P, 512], F32, tag="qk", name="mq")
                        for d in range(DT):
                            nc.tensor.matmul(
                                ps[:, 0:P],
                                lhsT=m16t[j_t // 2][:, j_t % 2, d, :],
                                rhs=xq16[:, d, :],
                                start=(d == 0),
                                stop=(d == DT - 1),
                            )
                        nc.scalar.copy(qMT16[:, j_t, :], ps[:, 0:P])
                    wts = apool.tile(
                        [P, KCH], BF16, tag="wts16", name="wts16", bufs=1
                    )
                    ps = qkps.tile([P, 512], F32, tag="qk", name="qk")
                    for d in range(DT):
                        nc.tensor.matmul(
                            ps[:, 0:KCH],
                            lhsT=qMT16[:, d, :],
                            rhs=xT16[:, d, :],
                            start=(d == 0),
                            stop=(d == DT - 1),
                        )
                    nc.vector.tensor_add(
                        ps[:, 0:KCH], ps[:, 0:KCH], msk[:, j, :]
                    )
                    nc.scalar.activation(
                        wts[:, 0:KCH],
                        ps[:, 0:KCH],
                        EXP,
                        scale=EXP_SCALE,
                        accum_out=st[:, 0:1],
                    )
                    nc.vector.reciprocal(st[:, 7:8], st[:, 0:1])
                    wT = wtpool.tile(
                        [P, 2, P], BF16, tag="wt16s", name="wt16s", bufs=1
                    )
                    nc.sync.dma_start(wT[:], wts[:, 0:KCH], transpose=True)
                    return st, wT

                groups = [(g * 512, 512) for g in range(C // 2)]
                if C % 2:
                    groups.append(((C // 2) * 512, 256))
                wts = apool.tile(
                    [P, S], BF16, tag="wts8", name="wts8", bufs=NSLOT - 1
                )
                nch = len(groups)
                for ci, (k0, ksz) in enumerate(groups):
                    ps = qkps.tile([P, 512], F32, tag="qk", name="qk")
                    for dp in range(DP):
                        nc.tensor.matmul(
                            ps[:, 0:ksz],
                            lhsT=qMT[:, 2 * dp : 2 * dp + 2, j * P : (j + 1) * P],
                            rhs=xTt[dp][:, :, k0 : k0 + ksz],
                            perf_mode=DR,
                            start=(dp == 0),
                            stop=(dp == DP - 1),
                        )
                    if k0 + ksz == L:
                        # causal edge: host mask covers the last 256 keys
                        nc.vector.tensor_add(
                            ps[:, ksz - 256 : ksz],
                            ps[:, ksz - 256 : ksz],
                            msk[:, j, :],
                        )
                    nc.scalar.activation(
                        wts[:, k0 : k0 + ksz],
                        ps[:, 0:ksz],
                        EXP,
                        scale=EXP_SCALE,
                        accum_out=st[:, ci : ci + 1],
                    )
                if nch > 1:
                    nc.vector.tensor_reduce(
                        st[:, 6:7], st[:, 0:nch], axis=AX, op=mybir.AluOpType.add
                    )
                    nc.vector.reciprocal(st[:, 7:8], st[:, 6:7])
                else:
                    nc.vector.reciprocal(st[:, 7:8], st[:, 0:1])
                if j in (1,):
                    # first pass-A slot: their xbar transposes would be
                    # queue-blocked behind the input DMA chains until ~47us;
                    # PE pair-transposes in pass A are cheaper than the stall
                    return st, wts
                wTb = wtpool.tile(
                    [P, nkb, P], BF16, tag=f"wtb{j}", name="wtb", bufs=1
                )
                nc.sync.dma_start(wTb[:], wts[:, 0:L], transpose=True)
                wT = wtpool.tile(
                    [P, nkb, P], FP8, tag=f"wt8{j}", name="wt8", bufs=1
                )
                nc.vector.tensor_copy(wT[:], wTb[:])
                return st, wT

            # emission order tucks slot 0 (whose bf16 mini-qproj + exp is the
            # longest dependency chain) mid-QK so nothing pends at the PSUM
            # pool transition; pass A/B still consume longest-first.
            QK_ORDER = [1, 3, 5, 0, 7, 6, 4, 2]
            with tc.tile_pool(name="qkps", bufs=4, space="PSUM") as qkps:
                scored = {jj: emit_scores(jj) for jj in QK_ORDER}
            staged = [(jj, *scored[jj]) for jj in SLOT_ORDER]

            # ---- PV pass A: w^T @ x8 in fp8 DoubleRow (slot 0 bf16).
            # Slots 1/3 transpose weight-block pairs on the PE here (their
            # xbar DMAs would be queue-blocked); the rest arrive
            # pre-transposed via xbar. (wx) transposes also via xbar; the
            # host packing of WvT matches the d*128+p blocked layout.
            wxT_all = []
            ident = c1pool.tile([P, P], BF16, tag="ident")
            make_identity(nc, ident[:])

            with (
                tc.tile_pool(name="wxps", bufs=6, space="PSUM") as wxps,
                tc.tile_pool(name="trps", bufs=2, space="PSUM") as trps,
            ):
                for si, (j, st, wT) in enumerate(staged):
                    nkb = CJ[j] * KCH // P
                    po = [
                        wxps.tile([P, 512], F32, tag="wx", name=f"wx{ec}")
                        for ec in range(2)
                    ]
                    if CJ[j] == 1:
                        for kb in range(nkb):
                            for ec in range(2):
                                nc.tensor.matmul(
                                    po[ec][:],
                                    lhsT=wT[:, kb, :],
                                    rhs=xn16[:, kb, ec * 512 : (ec + 1) * 512],
                                    start=(kb == 0),
                                    stop=(kb == nkb - 1),
                                )
                    elif j in (1,):
                        # wT here is the raw weight tile: PE pair-transposes
                        npair = nkb // 2
                        wTq = []

                        def emit_trp(kbp, wts=wT):
                            wTp = wtpool.tile(
                                [P, 2, P], FP8, tag="wTp", name="wTp", bufs=4
                            )
                            for i in range(2):
                                kb = 2 * kbp + i
                                pt = trps.tile(
                                    [P, P], BF16, tag="tr", name="pt"
                                )
                                nc.tensor.transpose(
                                    pt[:],
                                    wts[:, kb * P : (kb + 1) * P],
                                    ident[:],
                                )
                                nc.vector.tensor_copy(wTp[:, i, :], pt[:])
                            wTq.append(wTp)

                        emit_trp(0)
                        if npair > 1:
                            emit_trp(1)
                        for kbp in range(npair):
                            if kbp + 2 < npair:
                                emit_trp(kbp + 2)
                            for ec in range(2):
                                nc.tensor.matmul(
                                    po[ec][:],
                                    lhsT=wTq[kbp][:, 0:2, :],
                                    rhs=xnt[kbp // 2][
                                        :,
                                        (2 * kbp) % 4 : (2 * kbp) % 4 + 2,
                                        ec * 512 : (ec + 1) * 512,
                                    ],
                                    perf_mode=DR,
                                    start=(kbp == 0),
                                    stop=(kbp == npair - 1),
                                )
                    else:
                        npair = nkb // 2
                        for kbp in range(npair):
                            for ec in range(2):
                                nc.tensor.matmul(
                                    po[ec][:],
                                    lhsT=wT[:, 2 * kbp : 2 * kbp + 2, :],
                                    rhs=xnt[kbp // 2][
                                        :,
                                        (2 * kbp) % 4 : (2 * kbp) % 4 + 2,
                                        ec * 512 : (ec + 1) * 512,
                                    ],
                                    perf_mode=DR,
                                    start=(kbp == 0),
                                    stop=(kbp == npair - 1),
                                )
                    wx_sb = apool.tile(
                        [P, E], BF16, tag="wx", name="wx_sb", bufs=3
                    )
                    nc.scalar.copy(wx_sb[:, 0:512], po[0][:])
                    nc.vector.tensor_copy(wx_sb[:, 512:1024], po[1][:])
                    wxT = wxtpool.tile([P, DT, P], BF16, tag="wxT", name="wxT")
                    nc.sync.dma_start(wxT[:], wx_sb[:], transpose=True)
                    wxT_all.append(wxT)

                # ---- PV pass B: (wx)^T @ Wv^T, scaled by 1/sum, DMA out.
                # Same PSUM pool as pass A: a separate pool scope would put a
                # full PE drain barrier at the phase boundary.
                for si, (j, st, _) in enumerate(staged):
                    wxT = wxT_all[si]
                    po = [
                        wxps.tile([P, 512], F32, tag="wx", name=f"po{ec}")
                        for ec in range(2)
                    ]
                    for d in range(DT):
                        for ec in range(2):
                            nc.tensor.matmul(
                                po[ec][:],
                                lhsT=wxT[:, d, :],
                                rhs=wvt[d // 2][:, d % 2, ec * 512 : (ec + 1) * 512],
                                start=(d == 0),
                                stop=(d == DT - 1),
                            )
                    ot = apool.tile([P, E], F32, tag="out", name="ot")
                    nc.scalar.mul(ot[:, 0:512], po[0][:], st[:, 7:8])
                    nc.vector.tensor_scalar_mul(
                        ot[:, 512:1024], po[1][:], st[:, 7:8]
                    )
                    nc.sync.dma_start(out_d[j * P : (j + 1) * P, :], ot[:])



    nc.compile()
    return nc


_NC_CACHE = None


def _get_nc():
    global _NC_CACHE
    if _NC_CACHE is None:
        _NC_CACHE = build_kernel()
    return _NC_CACHE


def _pack_inputs(x, Wq, Wk, Wv):
    """Host-side relayout + weight folding."""
    bf = ml_dtypes.bfloat16
    f8 = ml_dtypes.float8_e4m3

    def to8(a):
        return np.clip(a, -240.0, 240.0).astype(f8)

    # folded scores matrix: scores = xq @ M @ xk^T with M = 32*(Wq^T @ Wk)
    # (the 32 re-centers fp8 quantization; the exp scale absorbs it).
    # packed for the q-projection lhsT: mp[p, jt, d, el] = M[d*128+p, jt*128+el]
    M32 = 32.0 * (
        Wq.T.astype(np.float64) @ Wk.astype(np.float64)
    ).astype(np.float32)
    m4 = M32.reshape(DT, P, DT, P).transpose(1, 2, 0, 3)
    mp = np.ascontiguousarray(to8(m4))
    mp16 = np.ascontiguousarray(m4.astype(bf))
    # Wv packed d-outer to match the xbar-transposed (wx)^T layout:
    # wxT[p, d, q] = wx[q, d*128+p], so [p, d, e] = Wv[e, d*128+p]
    wvp = np.ascontiguousarray(
        Wv.reshape(E, DT, P).transpose(2, 1, 0).astype(bf)
    )

    # causal masks per slot (identical formula for both cores' block lists)
    def packmask(blocks):
        m = np.zeros((NSLOT, P, KCH), np.float32)
        for j, blk in enumerate(blocks):
            cc = np.arange(KCH)[None, :] + (CJ[j] - 1) * KCH  # key col
            rr = np.arange(P)[:, None] + blk * P              # query row
            m[j] = np.where(cc <= rr, 0.0, MASK_VAL)
        return np.ascontiguousarray(m.transpose(1, 0, 2).astype(bf))  # [P,slot,KCH]

    masks = [packmask(QBLOCKS[0]), packmask(QBLOCKS[1])]

    in_maps = []
    for c in range(N_CORES):
        b, h = divmod(c, 2)
        xb = x[b]  # [S, D]
        xt = np.ascontiguousarray(
            to8(xb).reshape(S, DT, P).transpose(2, 1, 0)
        )
        xnat = np.ascontiguousarray(
            to8(xb).reshape(S // P, P, D).transpose(1, 0, 2)
        )
        xn16 = np.ascontiguousarray(
            xb[: 2 * P].reshape(2, P, D).transpose(1, 0, 2).astype(bf)
        )
        xt16 = np.ascontiguousarray(
            xb[:KCH].reshape(KCH, DT, P).transpose(2, 1, 0).astype(bf)
        )
        rows = np.concatenate(
            [np.arange(blk * P, (blk + 1) * P) for blk in QBLOCKS[h]]
        )
        xq = xb[rows]  # [SQ, D]
        xqt = np.ascontiguousarray(
            to8(xq).reshape(SQ, DT, P).transpose(2, 1, 0)
        )
        xq16 = np.ascontiguousarray(
            xq[:P].reshape(P, DT, P).transpose(2, 1, 0).astype(bf)
        )
        in_maps.append(
            {
                "xT": xt,
                "xn": xnat,
                "xqT": xqt,
                "MT": mp,
                "WvT": wvp,
                "masks": masks[h],
                "MT16": mp16,
                "xq16": xq16,
                "xT16": xt16,
                "xn16": xn16,
            }
        )
    return in_maps


def kernel(x, Wq, Wk, Wv, _spmd_kwargs=None, _results_out=None):
    x = np.asarray(x, dtype=np.float32)
    Wq = np.asarray(Wq, dtype=np.float32)
    Wk = np.asarray(Wk, dtype=np.float32)
    Wv = np.asarray(Wv, dtype=np.float32)
    assert x.shape == (B, S, D)

    nc = _get_nc()
    in_maps = _pack_inputs(x, Wq, Wk, Wv)
    res = run_bass_kernel_spmd(
        nc, in_maps, list(range(N_CORES)), **(_spmd_kwargs or {})
    )
    if _results_out is not None:
        _results_out.append(res)

    out = np.empty((B, S, E), np.float32)
    for c in range(N_CORES):
        b, h = divmod(c, 2)
        o = res.results[c]["out"]
        for j, blk in enumerate(QBLOCKS[h]):
            out[b, blk * P : (blk + 1) * P, :] = o[j * P : (j + 1) * P, :]
    return out
